# revision 49
# baseline (speedup 1.0000x reference)
"""Causal self-attention on 8 TRN2 NeuronCores.

Sharding: core c handles batch b=c//2, head-group g=c%2 (heads g*8..g*8+7).
Each core computes the qkv projection for its 8 heads, causal attention, and
a partial out-projection (its heads' columns of w_out). Host sums the two
partial outputs per batch. All layout transposes are done host-side.

On-chip (per core), P=128 partitions, bf16 matmul operands, f32 PSUM:
  xT    [1024(c), 2048(t)]   x[b] transposed
  wqkvT [1024(c), 1536(f)]   f = [qT 512 | kT 512 | vT 512] for this group
  woutT [512(dv), 1024(o)]   w_out columns for this group, transposed
  qk_sb f-tile m holds q (or k) for heads (2m, 2m+1) at partition offsets
  0/64, so the per-head QK^T matmuls (K=64) run as concurrent PE row-tiles
  T0/T8 (tile_position (0,0)/(64,0)): head pairs stream together, halving
  the score phase.  scoresT[j,i] psum tiles hold the pair side by side
  [A 512 | B 512]; one merged exp (ACT, scale=1/8) skips the not-computed
  sub-diagonal columns, and causal masking is a 0/1 bf16 multiply on the
  exp'd SBUF tile (4x DVE mode) of just the diagonal 128-blocks.
  PV accumulates per head into [65,512] psum via a ones column appended to
  v (row 64 = softmax denominator).  Normalization: in-place
  reciprocal_approx_fast on the psum den row, cast to bf16, then a K=33
  selector matmul broadcasts both heads' 1/den across partitions in one PE
  op; two fused psum-read multiplies write the normalized oT directly.
  Attention runs in 2-j-tile runs with PV lagging one run; QKV for block
  ib+1 and (during ib=3) the out-projection interleave as PE filler.
"""

import math
import numpy as np
import ml_dtypes

B, T, D, H, HD = 4, 2048, 1024, 16, 64
P = 128
HPG = 8          # heads per group
FG = HPG * HD    # 512 features per group
NCC = D // P     # 8 contraction chunks
NTB = 4          # t-blocks of 512
NTT = 16         # t-tiles of 128
NIB = 4          # i-blocks of 512
NPAIR = 4        # head pairs per group
SCALE = 1.0 / math.sqrt(HD)

_CACHE = {}


def _import_concourse():
    """Make concourse importable in environments where it isn't on sys.path."""
    try:
        import concourse.bass  # noqa: F401
        return
    except ImportError:
        pass
    import sys, os
    for p in ("/opt/trn_rl_repo", "/root/.axon_site/_ro/trn_rl_repo"):
        if os.path.isdir(p) and p not in sys.path:
            sys.path.insert(0, p)
    import concourse.bass  # noqa: F401


def _build_nc(debug_out=False):
    _import_concourse()
    from concourse import bacc
    import concourse.mybir as mybir
    import concourse.tile as tile
    from contextlib import ExitStack

    BF = mybir.dt.bfloat16
    F32 = mybir.dt.float32

    nc = bacc.Bacc("TRN2", target_bir_lowering=False, debug=False, num_devices=8)
    xT = nc.dram_tensor("xT", [D, T], BF, kind="ExternalInput").ap()
    wqkvT = nc.dram_tensor("wqkvT", [D, 3 * FG], BF, kind="ExternalInput").ap()
    woutT = nc.dram_tensor("woutT", [FG, D], BF, kind="ExternalInput").ap()
    mask2 = nc.dram_tensor("mask2", [P, 2, P], BF, kind="ExternalInput").ap()
    seld = nc.dram_tensor("seld", [33, P], F32, kind="ExternalInput").ap()
    out = nc.dram_tensor("out", [T, D], BF, kind="ExternalOutput").ap()
    if debug_out:
        qdbg = nc.dram_tensor("qdbg", [P, 8, T], BF, kind="ExternalOutput").ap()
        vdbg = nc.dram_tensor(
            "vdbg", [P, NTT, HPG, HD + 1], BF, kind="ExternalOutput").ap()
        odbg = nc.dram_tensor("odbg", [P, NPAIR, T], BF, kind="ExternalOutput").ap()
        ddbg = nc.dram_tensor(
            "ddbg", [NIB, NPAIR, 33, 512], F32, kind="ExternalOutput").ap()
        bdbg = nc.dram_tensor(
            "bdbg", [NIB, NPAIR, P, 512], BF, kind="ExternalOutput").ap()

    with tile.TileContext(nc) as tc, ExitStack() as ctx:
        singles = ctx.enter_context(tc.tile_pool(name="singles", bufs=1))
        xtp = ctx.enter_context(tc.tile_pool(name="xt", bufs=2))
        ptp = ctx.enter_context(tc.tile_pool(name="pt", bufs=10))
        bcp = ctx.enter_context(tc.tile_pool(name="bc", bufs=3))
        yp = ctx.enter_context(tc.tile_pool(name="y", bufs=3))
        ps_qk = ctx.enter_context(tc.tile_pool(name="ps_qk", bufs=2, space="PSUM"))
        ps_pv = ctx.enter_context(tc.tile_pool(name="ps_pv", bufs=2, space="PSUM"))
        ps_mm = ctx.enter_context(tc.tile_pool(name="ps_mm", bufs=2, space="PSUM"))

        wq_sb = singles.tile([P, NCC, 3 * FG], BF)
        wq_src = wqkvT.rearrange("(cc p) f -> p cc f", p=P)
        # q/k f-tiles land one DMA each in pair-usage order.  The x block-0
        # DMAs are emitted between ft0/ft4 and the rest (inside
        # emit_qkv_block(0) below) so the first matmul group completes
        # ~5us in; the v slab follows for pair 0's PV.
        for ft in (0, 4):
            nc.sync.dma_start(
                out=wq_sb[:, :, ft * P:(ft + 1) * P],
                in_=wq_src[:, :, ft * P:(ft + 1) * P],
            )

        def dma_wq_rest():
            nc.sync.dma_start(
                out=wq_sb[:, :, 2 * FG:3 * FG], in_=wq_src[:, :, 2 * FG:3 * FG]
            )
            for ft in (1, 5, 2, 6, 3, 7):
                nc.sync.dma_start(
                    out=wq_sb[:, :, ft * P:(ft + 1) * P],
                    in_=wq_src[:, :, ft * P:(ft + 1) * P],
                )
        mask_sb = singles.tile([P, 2, P], BF)
        nc.sync.dma_start(out=mask_sb, in_=mask2)
        sel_sb = singles.tile([33, P], F32)
        nc.sync.dma_start(out=sel_sb, in_=seld)
        wo_sb = singles.tile([P, 4, D], BF)

        qk_sb = singles.tile([P, 8, T], BF)              # f-tiles 0..3 = q, 4..7 = k
        vp_sb = singles.tile([P, NTT, HPG, HD + 1], BF)  # [v_h | ones]
        oT_sb = singles.tile([P, NPAIR, T], BF)          # attn out, [dv-pair, t]
        dp = singles.tile([33, 512], F32)                # den rows 0 (A), 32 (B)
        dpr = singles.tile([33, 512], F32)               # 1/den
        nc.vector.memset(vp_sb[:, :, :, HD:HD + 1], 1.0)
        nc.vector.memset(dp, 1.0)

        # ---- filler thunks: qkv blocks + out-projection ----
        def emit_qkv_block(tb):
            """DMA x-block tb, return one thunk per psum group.
            Order: q/k tiles in pair-usage order, then v tiles."""
            xt = xtp.tile([P, NCC, 512], BF)
            xt_src = xT[:, tb * 512:(tb + 1) * 512].rearrange(
                "(cc p) t -> p cc t", p=P)
            # two half-block DMAs: cheap to issue, and cc 0-3 matmuls can
            # start while cc 4-7 are in flight
            nc.sync.dma_start(out=xt[:, 0:4, :], in_=xt_src[:, 0:4, :])
            nc.sync.dma_start(out=xt[:, 4:8, :], in_=xt_src[:, 4:8, :])
            thunks = []
            for ft in range(8):  # q then k feature tiles, output [f=128, t=512]
                def qk_group(ft=ft, xt=xt, tb=tb):  # key ("qk", tb, ft)
                    ps = ps_mm.tile([P, 512], F32)
                    for cc in range(NCC):
                        nc.tensor.matmul(
                            ps,
                            lhsT=wq_sb[:, cc, ft * P:(ft + 1) * P],
                            rhs=xt[:, cc, :],
                            start=(cc == 0),
                            stop=(cc == NCC - 1),
                        )
                    nc.vector.tensor_copy(
                        out=qk_sb[:, ft, tb * 512:(tb + 1) * 512], in_=ps
                    )
                thunks.append((("qk", tb, ft), qk_group))
            for tl in range(4):  # v in [t, dv] orientation, output [t=128, dv=512]
                def v_group(tl=tl, xt=xt, tb=tb):
                    tt = tb * 4 + tl
                    ps = ps_mm.tile([P, FG], F32)
                    for cc in range(NCC):
                        nc.tensor.matmul(
                            ps,
                            lhsT=xt[:, cc, tl * P:(tl + 1) * P],
                            rhs=wq_sb[:, cc, 2 * FG:3 * FG],
                            start=(cc == 0),
                            stop=(cc == NCC - 1),
                        )
                    nc.vector.tensor_copy(
                        out=vp_sb[:, tt, :, 0:HD],
                        in_=ps.rearrange("p (h d) -> p h d", h=HPG),
                    )
                thunks.append((("v", tb * 4 + tl), v_group))
            # interleave q/k pair-wise: q0,k0,q1,k1,... then v0..v3
            order = [0, 4, 1, 5, 2, 6, 3, 7, 8, 9, 10, 11]
            return [thunks[i] for i in order]

        def emit_outproj_tt(tt, tail=False):
            yt = yp.tile([P, 1024], BF)
            for ob in range(2):
                ps = ps_mm.tile([P, 512], F32, tag="ps", name="ps_op")
                for dc in range(4):
                    nc.tensor.matmul(
                        ps,
                        lhsT=oT_sb[:, dc, tt * P:(tt + 1) * P],
                        rhs=wo_sb[:, dc, ob * 512:(ob + 1) * 512],
                        start=(dc == 0),
                        stop=(dc == 3),
                    )
                if tail:  # ACT is idle in the tail; halve the copy chain
                    nc.scalar.copy(out=yt[:, ob * 512:(ob + 1) * 512], in_=ps)
                else:
                    nc.vector.tensor_copy(yt[:, ob * 512:(ob + 1) * 512], ps)
            nc.sync.dma_start(out=out[tt * P:(tt + 1) * P, :], in_=yt)

        # ---- attention ----
        def emit_attn_pair(ib, m, pop_filler, need, finish_prev):
            """Scores+softmax+PV for head pair (2m, 2m+1), i-block ib.
            pop_filler() emits paced filler; need(keys) force-drains filler
            thunks this pair reads from; finish_prev finishes the previous
            pair's normalization (emitted inside the 64-mode QK runs)."""
            njt = 4 * ib + 4
            isl = slice(ib * 512, (ib + 1) * 512)
            fq, fk = m, 4 + m
            pvA = ps_pv.tile([HD + 1, 512], F32, tag="pv", name="pvA")
            pvB = ps_pv.tile([HD + 1, 512], F32, tag="pv", name="pvB")
            pts = {}

            def qk_run(jts, extra=None):
                for jt in jts:
                    r = jt - 4 * ib
                    c0 = P * r if r > 0 else 0
                    ps = ps_qk.tile([P, 1024], F32)
                    nc.tensor.matmul(
                        ps[:, c0:512],
                        lhsT=qk_sb[0:64, fk, jt * P:(jt + 1) * P],
                        rhs=qk_sb[0:64, fq, ib * 512 + c0:(ib + 1) * 512],
                        start=True, stop=True,
                    )
                    nc.tensor.matmul(
                        ps[:, 512 + c0:1024],
                        lhsT=qk_sb[64:128, fk, jt * P:(jt + 1) * P],
                        rhs=qk_sb[64:128, fq, ib * 512 + c0:(ib + 1) * 512],
                        start=True, stop=True,
                    )
                    if extra is not None:  # same-mode (64,128) deferred work
                        extra(); extra = None
                    pt = ptp.tile([P, 1024], BF)
                    ps2 = ps.rearrange("p (g w) -> p g w", g=2)
                    pt2 = pt.rearrange("p (g w) -> p g w", g=2)
                    nc.scalar.activation(
                        out=pt2[:, :, c0:512], in_=ps2[:, :, c0:512],
                        func=mybir.ActivationFunctionType.Exp, scale=SCALE,
                    )
                    if r >= 0:  # zero the upper triangle of the diagonal block
                        nc.vector.tensor_mul(
                            pt2[:, :, c0:c0 + P], pt2[:, :, c0:c0 + P], mask_sb
                        )
                    pts[jt] = (pt, c0)
                if extra is not None:
                    extra()

            def pv_run(jts):
                for jt in jts:
                    pt, c0 = pts.pop(jt)
                    pt2 = pt.rearrange("p (g w) -> p g w", g=2)
                    first, last = (jt == 0), (jt == njt - 1)
                    nc.tensor.matmul(
                        pvA[:, c0:512],
                        lhsT=vp_sb[:, jt, 2 * m, :],
                        rhs=pt2[:, 0, c0:512],
                        start=first, stop=last, skip_group_check=True,
                    )
                    nc.tensor.matmul(
                        pvB[:, c0:512],
                        lhsT=vp_sb[:, jt, 2 * m + 1, :],
                        rhs=pt2[:, 1, c0:512],
                        start=first, stop=last, skip_group_check=True,
                    )

            need([("qk", tb, fk) for tb in range(ib + 1)] + [("qk", ib, fq)])
            runs = [list(range(s, min(s + 2, njt))) for s in range(0, njt, 2)]
            prev = None
            for ri, run in enumerate(runs):
                qk_run(run, extra=(finish_prev if ri == min(1, len(runs) - 1)
                                   else None))
                pop_filler()  # filler before PV: exps + normalize get slack
                if ri == 0:  # extra filler at the pair boundary, where the
                    pop_filler()  # previous pair's normalization needs slack
                if prev is not None:
                    need([("v", jt) for jt in prev])
                    pv_run(prev)
                prev = run
            need([("v", jt) for jt in prev])
            pv_run(prev)

            # start normalization: gather dens to SBUF, 1/den, cast to bf16
            nc.vector.tensor_copy(out=dp[0:1, :], in_=pvA[HD:HD + 1, :])
            nc.vector.tensor_copy(out=dp[32:33, :], in_=pvB[HD:HD + 1, :])
            nc.vector.reciprocal_approx_fast(out=dpr, in_=dp)
            if debug_out:
                nc.sync.dma_start(out=ddbg[ib, m], in_=dpr)

            def finish():
                # broadcast 1/den across partitions with a K=33 selector
                # matmul (64-mode, emitted inside the next pair's QK run),
                # then two fused psum-read muls write normalized oT.
                bc_ps = ps_mm.tile([P, 512], F32, tag="ps", name="bc_ps")
                nc.tensor.matmul(bc_ps, lhsT=sel_sb, rhs=dpr,
                                 start=True, stop=True)
                bc = bcp.tile([P, 512], BF)
                nc.vector.tensor_copy(out=bc, in_=bc_ps)
                if debug_out:
                    nc.sync.dma_start(out=bdbg[ib, m], in_=bc)
                nc.vector.tensor_mul(
                    oT_sb[0:HD, m, isl], pvA[0:HD, :], bc[0:64, :])
                nc.vector.tensor_mul(
                    oT_sb[64:64 + HD, m, isl], pvB[0:HD, :], bc[64:128, :])
            return finish

        # ---- top-level emission ----
        filler = []          # list of (key, fn)
        fill_done = 0
        emitted = set()

        def drain_one():
            nonlocal fill_done
            key, fn = filler[fill_done]
            fn()
            emitted.add(key)
            fill_done += 1

        def need(keys):
            while not all(k in emitted for k in keys):
                drain_one()

        def make_pop(total_runs):
            state = {"run": 0, "base": fill_done}
            def pop():
                state["run"] += 1
                want = min(len(filler), state["base"] + (
                    (len(filler) - state["base"]) * state["run"] // total_runs))
                while fill_done < want:
                    drain_one()
            return pop

        filler += emit_qkv_block(0)   # emits the x block-0 DMAs
        dma_wq_rest()
        # pull the first q/k pair groups upfront so attention starts early
        drain_one(); drain_one()

        finish_prev = None
        for ib in range(NIB):
            if ib == 1:  # wo is needed from ib3; keep it off the hot window
                nc.sync.dma_start(
                    out=wo_sb, in_=woutT.rearrange("(dc p) o -> p dc o", p=P))
            if ib < 3:
                filler.extend(emit_qkv_block(ib + 1))
            else:
                # tts 0-7 read oT of ib0/ib1, fully normalized by now; tts
                # 8-11 (ib2 rows) need finish(ib2, pair3), emitted during
                # pair 0 below, so they join the filler list after it.
                filler.extend(
                    ((("op", tt), (lambda tt=tt: emit_outproj_tt(tt)))
                     for tt in range(8)))
            pop = make_pop(total_runs=NPAIR * (2 * ib + 2))
            for m in range(NPAIR):
                finish_prev = emit_attn_pair(ib, m, pop, need, finish_prev)
                if ib == 3 and m == 0:
                    filler.extend(
                        ((("op", tt), (lambda tt=tt: emit_outproj_tt(tt)))
                         for tt in range(8, 12)))
        finish_prev()
        while fill_done < len(filler):
            drain_one()
        for tt in range(12, 16):
            emit_outproj_tt(tt, tail=True)
        if debug_out:
            nc.sync.dma_start(out=qdbg, in_=qk_sb)
            nc.sync.dma_start(out=vdbg, in_=vp_sb)
            nc.sync.dma_start(out=odbg, in_=oT_sb)
    nc.compile()
    return nc


def _make_in_maps(x, w_qkv, w_out):
    bf = ml_dtypes.bfloat16
    # 0/1 mask for the diagonal 128x128 block: keep i_local >= j_local
    tri = (np.arange(P)[None, :] >= np.arange(P)[:, None]).astype(np.float32)
    mask2 = np.ascontiguousarray(
        np.broadcast_to(tri[:, None, :], (P, 2, P))).astype(bf)
    sel = np.zeros((33, P), dtype=np.float32)
    sel[0, 0:64] = 1.0
    sel[32, 64:128] = 1.0
    in_maps = []
    for c in range(8):
        b, g = c // 2, c % 2
        wq = w_qkv[g * FG:(g + 1) * FG]
        wk = w_qkv[D + g * FG:D + (g + 1) * FG]
        wv = w_qkv[2 * D + g * FG:2 * D + (g + 1) * FG]
        in_maps.append({
            "xT": np.ascontiguousarray(x[b].T).astype(bf),
            "wqkvT": np.ascontiguousarray(
                np.concatenate([wq.T, wk.T, wv.T], axis=1)).astype(bf),
            "woutT": np.ascontiguousarray(w_out[:, g * FG:(g + 1) * FG].T).astype(bf),
            "mask2": mask2,
            "seld": sel,
        })
    return in_maps


def _ensure_ntff_hook():
    """The agent image's antenv package lacks axon_hooks; shim it so
    run_bass_kernel_spmd(trace=True) can capture NTFF profiles."""
    import sys, types
    try:
        import antenv.axon_hooks  # noqa: F401
        return
    except ImportError:
        pass
    import antenv
    mod = types.ModuleType("antenv.axon_hooks")
    mod._hook = None
    def set_axon_ntff_profile_hook(h):
        mod._hook = h
    def get_axon_ntff_profile_hook():
        return mod._hook
    mod.set_axon_ntff_profile_hook = set_axon_ntff_profile_hook
    mod.get_axon_ntff_profile_hook = get_axon_ntff_profile_hook
    sys.modules["antenv.axon_hooks"] = mod
    antenv.axon_hooks = mod
    try:
        from trn_agent_boot.trn_boot import _ntff_profile_via_ctypes
        set_axon_ntff_profile_hook(
            _ntff_profile_via_ctypes("/opt/axon/libaxon_pjrt.so"))
    except Exception as e:  # degrade to no tracing
        print(f"ntff hook install failed: {e}")


def run(x, w_qkv, w_out, trace=False, trace_kwargs=None, debug=False):
    _import_concourse()
    if trace:
        _ensure_ntff_hook()
    from concourse.bass_utils import run_bass_kernel_spmd

    key = "nc_dbg" if debug else "nc"
    if key not in _CACHE:
        _CACHE[key] = _build_nc(debug_out=debug)
    nc = _CACHE[key]
    in_maps = _make_in_maps(np.asarray(x), np.asarray(w_qkv), np.asarray(w_out))
    kw = dict(trace_kwargs or {})
    res = run_bass_kernel_spmd(nc, in_maps, core_ids=list(range(8)), trace=trace, **kw)
    outs = [r["out"] for r in res.results]
    full = np.empty((B, T, D), dtype=np.float32)
    for b in range(B):
        full[b] = outs[2 * b].astype(np.float32) + outs[2 * b + 1].astype(np.float32)
    return full, res


def kernel(x, w_qkv, w_out):
    full, _ = run(x, w_qkv, w_out, trace=False)
    return full


# revision 50
# speedup vs baseline: 1.0122x; 1.0122x over previous
"""Causal self-attention on 8 TRN2 NeuronCores.

Sharding: core c handles batch b=c//2, head-group g=c%2 (heads g*8..g*8+7).
Each core computes the qkv projection for its 8 heads, causal attention, and
a partial out-projection (its heads' columns of w_out). Host sums the two
partial outputs per batch. All layout transposes are done host-side.

On-chip (per core), P=128 partitions, bf16 matmul operands, f32 PSUM:
  xT    [1024(c), 2048(t)]   x[b] transposed
  wqkvT [1024(c), 1536(f)]   f = [qT 512 | kT 512 | vT 512] for this group
  woutT [512(dv), 1024(o)]   w_out columns for this group, transposed
  qk_sb f-tile m holds q (or k) for heads (2m, 2m+1) at partition offsets
  0/64, so the per-head QK^T matmuls (K=64) run as concurrent PE row-tiles
  T0/T8 (tile_position (0,0)/(64,0)): head pairs stream together, halving
  the score phase.  scoresT[j,i] psum tiles hold the pair side by side
  [A 512 | B 512]; one merged exp (ACT, scale=1/8) skips the not-computed
  sub-diagonal columns, and causal masking is a 0/1 bf16 multiply on the
  exp'd SBUF tile (4x DVE mode) of just the diagonal 128-blocks.
  PV accumulates per head into [65,512] psum via a ones column appended to
  v (row 64 = softmax denominator).  Normalization: in-place
  reciprocal_approx_fast on the psum den row, cast to bf16, then a K=33
  selector matmul broadcasts both heads' 1/den across partitions in one PE
  op; two fused psum-read multiplies write the normalized oT directly.
  Attention runs in 2-j-tile runs with PV lagging one run; QKV for block
  ib+1 and (during ib=3) the out-projection interleave as PE filler.
"""

import math
import numpy as np
import ml_dtypes

B, T, D, H, HD = 4, 2048, 1024, 16, 64
P = 128
HPG = 8          # heads per group
FG = HPG * HD    # 512 features per group
NCC = D // P     # 8 contraction chunks
NTB = 4          # t-blocks of 512
NTT = 16         # t-tiles of 128
NIB = 4          # i-blocks of 512
NPAIR = 4        # head pairs per group
SCALE = 1.0 / math.sqrt(HD)

_CACHE = {}


def _import_concourse():
    """Make concourse importable in environments where it isn't on sys.path."""
    try:
        import concourse.bass  # noqa: F401
        return
    except ImportError:
        pass
    import sys, os
    for p in ("/opt/trn_rl_repo", "/root/.axon_site/_ro/trn_rl_repo"):
        if os.path.isdir(p) and p not in sys.path:
            sys.path.insert(0, p)
    import concourse.bass  # noqa: F401


def _build_nc(debug_out=False):
    _import_concourse()
    from concourse import bacc
    import concourse.mybir as mybir
    import concourse.tile as tile
    from contextlib import ExitStack

    BF = mybir.dt.bfloat16
    F32 = mybir.dt.float32

    nc = bacc.Bacc("TRN2", target_bir_lowering=False, debug=False, num_devices=8)
    xT = nc.dram_tensor("xT", [D, T], BF, kind="ExternalInput").ap()
    wqkvT = nc.dram_tensor("wqkvT", [D, 3 * FG], BF, kind="ExternalInput").ap()
    woutT = nc.dram_tensor("woutT", [FG, D], BF, kind="ExternalInput").ap()
    mask2 = nc.dram_tensor("mask2", [P, 2, P], BF, kind="ExternalInput").ap()
    seld = nc.dram_tensor("seld", [33, P], F32, kind="ExternalInput").ap()
    out = nc.dram_tensor("out", [T, D], BF, kind="ExternalOutput").ap()
    if debug_out:
        qdbg = nc.dram_tensor("qdbg", [P, 8, T], BF, kind="ExternalOutput").ap()
        vdbg = nc.dram_tensor(
            "vdbg", [P, NTT, HPG, HD + 1], BF, kind="ExternalOutput").ap()
        odbg = nc.dram_tensor("odbg", [P, NPAIR, T], BF, kind="ExternalOutput").ap()
        ddbg = nc.dram_tensor(
            "ddbg", [NIB, NPAIR, 33, 512], F32, kind="ExternalOutput").ap()
        bdbg = nc.dram_tensor(
            "bdbg", [NIB, NPAIR, P, 512], BF, kind="ExternalOutput").ap()

    with tile.TileContext(nc) as tc, ExitStack() as ctx:
        singles = ctx.enter_context(tc.tile_pool(name="singles", bufs=1))
        xtp = ctx.enter_context(tc.tile_pool(name="xt", bufs=2))
        ptp = ctx.enter_context(tc.tile_pool(name="pt", bufs=10))
        bcp = ctx.enter_context(tc.tile_pool(name="bc", bufs=3))
        yp = ctx.enter_context(tc.tile_pool(name="y", bufs=3))
        ps_qk = ctx.enter_context(tc.tile_pool(name="ps_qk", bufs=2, space="PSUM"))
        ps_pv = ctx.enter_context(tc.tile_pool(name="ps_pv", bufs=2, space="PSUM"))
        ps_mm = ctx.enter_context(tc.tile_pool(name="ps_mm", bufs=2, space="PSUM"))

        wq_sb = singles.tile([P, NCC, 3 * FG], BF)
        wq_src = wqkvT.rearrange("(cc p) f -> p cc f", p=P)
        # q/k f-tiles land one DMA each in pair-usage order.  The x block-0
        # DMAs are emitted between ft0/ft4 and the rest (inside
        # emit_qkv_block(0) below) so the first matmul group completes
        # ~5us in; the v slab follows for pair 0's PV.
        for ft in (0, 4):
            nc.sync.dma_start(
                out=wq_sb[:, :, ft * P:(ft + 1) * P],
                in_=wq_src[:, :, ft * P:(ft + 1) * P],
            )

        def dma_wq_rest():
            nc.sync.dma_start(
                out=wq_sb[:, :, 2 * FG:3 * FG], in_=wq_src[:, :, 2 * FG:3 * FG]
            )
            for ft in (1, 5, 2, 6, 3, 7):
                nc.sync.dma_start(
                    out=wq_sb[:, :, ft * P:(ft + 1) * P],
                    in_=wq_src[:, :, ft * P:(ft + 1) * P],
                )
        mask_sb = singles.tile([P, 2, P], BF)
        nc.sync.dma_start(out=mask_sb, in_=mask2)
        sel_sb = singles.tile([33, P], F32)
        nc.sync.dma_start(out=sel_sb, in_=seld)
        wo_sb = singles.tile([P, 4, D], BF)

        qk_sb = singles.tile([P, 8, T], BF)              # f-tiles 0..3 = q, 4..7 = k
        vp_sb = singles.tile([P, NTT, HPG, HD + 1], BF)  # [v_h | ones]
        oT_sb = singles.tile([P, NPAIR, T], BF)          # attn out, [dv-pair, t]
        dp = singles.tile([33, 512], F32)                # den rows 0 (A), 32 (B)
        dpr = singles.tile([33, 512], F32)               # 1/den
        nc.vector.memset(vp_sb[:, :, :, HD:HD + 1], 1.0)
        nc.vector.memset(dp, 1.0)

        # ---- filler thunks: qkv blocks + out-projection ----
        def emit_qkv_block(tb):
            """DMA x-block tb, return one thunk per psum group.
            Order: q/k tiles in pair-usage order, then v tiles."""
            xt = xtp.tile([P, NCC, 512], BF)
            xt_src = xT[:, tb * 512:(tb + 1) * 512].rearrange(
                "(cc p) t -> p cc t", p=P)
            # two half-block DMAs: cheap to issue, and cc 0-3 matmuls can
            # start while cc 4-7 are in flight
            nc.sync.dma_start(out=xt[:, 0:4, :], in_=xt_src[:, 0:4, :])
            nc.sync.dma_start(out=xt[:, 4:8, :], in_=xt_src[:, 4:8, :])
            thunks = []
            for ft in range(8):  # q then k feature tiles, output [f=128, t=512]
                def qk_group(ft=ft, xt=xt, tb=tb):  # key ("qk", tb, ft)
                    ps = ps_mm.tile([P, 512], F32)
                    for cc in range(NCC):
                        nc.tensor.matmul(
                            ps,
                            lhsT=wq_sb[:, cc, ft * P:(ft + 1) * P],
                            rhs=xt[:, cc, :],
                            start=(cc == 0),
                            stop=(cc == NCC - 1),
                        )
                    nc.vector.tensor_copy(
                        out=qk_sb[:, ft, tb * 512:(tb + 1) * 512], in_=ps
                    )
                thunks.append((("qk", tb, ft), qk_group))
            for tl in range(4):  # v in [t, dv] orientation, output [t=128, dv=512]
                def v_group(tl=tl, xt=xt, tb=tb):
                    tt = tb * 4 + tl
                    ps = ps_mm.tile([P, FG], F32)
                    for cc in range(NCC):
                        nc.tensor.matmul(
                            ps,
                            lhsT=xt[:, cc, tl * P:(tl + 1) * P],
                            rhs=wq_sb[:, cc, 2 * FG:3 * FG],
                            start=(cc == 0),
                            stop=(cc == NCC - 1),
                        )
                    nc.vector.tensor_copy(
                        out=vp_sb[:, tt, :, 0:HD],
                        in_=ps.rearrange("p (h d) -> p h d", h=HPG),
                    )
                thunks.append((("v", tb * 4 + tl), v_group))
            # interleave q/k pair-wise: q0,k0,q1,k1,... then v0..v3
            order = [0, 4, 1, 5, 2, 6, 3, 7, 8, 9, 10, 11]
            return [thunks[i] for i in order]

        def emit_outproj_tt(tt, tail=False):
            yt = yp.tile([P, 1024], BF)
            for ob in range(2):
                ps = ps_mm.tile([P, 512], F32, tag="ps", name="ps_op")
                for dc in range(4):
                    nc.tensor.matmul(
                        ps,
                        lhsT=oT_sb[:, dc, tt * P:(tt + 1) * P],
                        rhs=wo_sb[:, dc, ob * 512:(ob + 1) * 512],
                        start=(dc == 0),
                        stop=(dc == 3),
                    )
                if tail:  # ACT is idle in the tail; halve the copy chain
                    nc.scalar.copy(out=yt[:, ob * 512:(ob + 1) * 512], in_=ps)
                else:
                    nc.vector.tensor_copy(yt[:, ob * 512:(ob + 1) * 512], ps)
            nc.sync.dma_start(out=out[tt * P:(tt + 1) * P, :], in_=yt)

        # ---- attention ----
        def emit_attn_pair(ib, m, pop_filler, need, finish_prev):
            """Scores+softmax+PV for head pair (2m, 2m+1), i-block ib.
            pop_filler() emits paced filler; need(keys) force-drains filler
            thunks this pair reads from; finish_prev finishes the previous
            pair's normalization (emitted inside the 64-mode QK runs)."""
            njt = 4 * ib + 4
            isl = slice(ib * 512, (ib + 1) * 512)
            fq, fk = m, 4 + m
            pvA = ps_pv.tile([HD + 1, 512], F32, tag="pv", name="pvA")
            pvB = ps_pv.tile([HD + 1, 512], F32, tag="pv", name="pvB")
            pts = {}

            def qk_run(jts, extra=None):
                for jt in jts:
                    r = jt - 4 * ib
                    c0 = P * r if r > 0 else 0
                    ps = ps_qk.tile([P, 1024], F32)
                    nc.tensor.matmul(
                        ps[:, c0:512],
                        lhsT=qk_sb[0:64, fk, jt * P:(jt + 1) * P],
                        rhs=qk_sb[0:64, fq, ib * 512 + c0:(ib + 1) * 512],
                        start=True, stop=True,
                    )
                    nc.tensor.matmul(
                        ps[:, 512 + c0:1024],
                        lhsT=qk_sb[64:128, fk, jt * P:(jt + 1) * P],
                        rhs=qk_sb[64:128, fq, ib * 512 + c0:(ib + 1) * 512],
                        start=True, stop=True,
                    )
                    if extra is not None:  # same-mode (64,128) deferred work
                        extra(); extra = None
                    pt = ptp.tile([P, 1024], BF)
                    ps2 = ps.rearrange("p (g w) -> p g w", g=2)
                    pt2 = pt.rearrange("p (g w) -> p g w", g=2)
                    nc.scalar.activation(
                        out=pt2[:, :, c0:512], in_=ps2[:, :, c0:512],
                        func=mybir.ActivationFunctionType.Exp, scale=SCALE,
                    )
                    if r >= 0:  # zero the upper triangle of the diagonal block
                        nc.vector.tensor_mul(
                            pt2[:, :, c0:c0 + P], pt2[:, :, c0:c0 + P], mask_sb
                        )
                    pts[jt] = (pt, c0)
                if extra is not None:
                    extra()

            def pv_run(jts):
                for jt in jts:
                    pt, c0 = pts.pop(jt)
                    pt2 = pt.rearrange("p (g w) -> p g w", g=2)
                    first, last = (jt == 0), (jt == njt - 1)
                    nc.tensor.matmul(
                        pvA[:, c0:512],
                        lhsT=vp_sb[:, jt, 2 * m, :],
                        rhs=pt2[:, 0, c0:512],
                        start=first, stop=last, skip_group_check=True,
                    )
                    nc.tensor.matmul(
                        pvB[:, c0:512],
                        lhsT=vp_sb[:, jt, 2 * m + 1, :],
                        rhs=pt2[:, 1, c0:512],
                        start=first, stop=last, skip_group_check=True,
                    )

            need([("qk", tb, fk) for tb in range(ib + 1)] + [("qk", ib, fq)])
            # longer runs in the ACT-bound last block: fewer PE mode
            # switches, and the QK->exp chain paces itself there anyway
            rsz = 2 if ib < 3 else 4
            runs = [list(range(s, min(s + rsz, njt)))
                    for s in range(0, njt, rsz)]
            prev = None
            for ri, run in enumerate(runs):
                qk_run(run, extra=(finish_prev if ri == min(1, len(runs) - 1)
                                   else None))
                pop_filler()  # filler before PV: exps + normalize get slack
                if ri == 0:  # extra filler at the pair boundary, where the
                    pop_filler()  # previous pair's normalization needs slack
                if prev is not None:
                    need([("v", jt) for jt in prev])
                    pv_run(prev)
                prev = run
            need([("v", jt) for jt in prev])
            pv_run(prev)

            # start normalization: gather dens to SBUF, 1/den, cast to bf16
            nc.vector.tensor_copy(out=dp[0:1, :], in_=pvA[HD:HD + 1, :])
            nc.vector.tensor_copy(out=dp[32:33, :], in_=pvB[HD:HD + 1, :])
            nc.vector.reciprocal_approx_fast(out=dpr, in_=dp)
            if debug_out:
                nc.sync.dma_start(out=ddbg[ib, m], in_=dpr)

            def finish():
                # broadcast 1/den across partitions with a K=33 selector
                # matmul (64-mode, emitted inside the next pair's QK run),
                # then two fused psum-read muls write normalized oT.
                bc_ps = ps_mm.tile([P, 512], F32, tag="ps", name="bc_ps")
                nc.tensor.matmul(bc_ps, lhsT=sel_sb, rhs=dpr,
                                 start=True, stop=True)
                bc = bcp.tile([P, 512], BF)
                nc.vector.tensor_copy(out=bc, in_=bc_ps)
                if debug_out:
                    nc.sync.dma_start(out=bdbg[ib, m], in_=bc)
                nc.vector.tensor_mul(
                    oT_sb[0:HD, m, isl], pvA[0:HD, :], bc[0:64, :])
                nc.vector.tensor_mul(
                    oT_sb[64:64 + HD, m, isl], pvB[0:HD, :], bc[64:128, :])
            return finish

        # ---- top-level emission ----
        filler = []          # list of (key, fn)
        fill_done = 0
        emitted = set()

        def drain_one():
            nonlocal fill_done
            key, fn = filler[fill_done]
            fn()
            emitted.add(key)
            fill_done += 1

        def need(keys):
            while not all(k in emitted for k in keys):
                drain_one()

        def make_pop(total_runs):
            state = {"run": 0, "base": fill_done}
            def pop():
                state["run"] += 1
                want = min(len(filler), state["base"] + (
                    (len(filler) - state["base"]) * state["run"] // total_runs))
                while fill_done < want:
                    drain_one()
            return pop

        filler += emit_qkv_block(0)   # emits the x block-0 DMAs
        dma_wq_rest()
        # pull the first q/k pair groups upfront so attention starts early
        drain_one(); drain_one()

        finish_prev = None
        for ib in range(NIB):
            if ib == 1:  # wo is needed from ib3; keep it off the hot window
                nc.sync.dma_start(
                    out=wo_sb, in_=woutT.rearrange("(dc p) o -> p dc o", p=P))
            if ib < 3:
                filler.extend(emit_qkv_block(ib + 1))
            else:
                # tts 0-7 read oT of ib0/ib1, fully normalized by now; tts
                # 8-11 (ib2 rows) need finish(ib2, pair3), emitted during
                # pair 0 below, so they join the filler list after it.
                filler.extend(
                    ((("op", tt), (lambda tt=tt: emit_outproj_tt(tt)))
                     for tt in range(8)))
            pop = make_pop(total_runs=NPAIR * (2 * ib + 2))
            for m in range(NPAIR):
                finish_prev = emit_attn_pair(ib, m, pop, need, finish_prev)
                if ib == 3 and m == 0:
                    filler.extend(
                        ((("op", tt), (lambda tt=tt: emit_outproj_tt(tt)))
                         for tt in range(8, 12)))
        finish_prev()
        while fill_done < len(filler):
            drain_one()
        for tt in range(12, 16):
            emit_outproj_tt(tt, tail=True)
        if debug_out:
            nc.sync.dma_start(out=qdbg, in_=qk_sb)
            nc.sync.dma_start(out=vdbg, in_=vp_sb)
            nc.sync.dma_start(out=odbg, in_=oT_sb)
    nc.compile()
    return nc


def _make_in_maps(x, w_qkv, w_out):
    bf = ml_dtypes.bfloat16
    # 0/1 mask for the diagonal 128x128 block: keep i_local >= j_local
    tri = (np.arange(P)[None, :] >= np.arange(P)[:, None]).astype(np.float32)
    mask2 = np.ascontiguousarray(
        np.broadcast_to(tri[:, None, :], (P, 2, P))).astype(bf)
    sel = np.zeros((33, P), dtype=np.float32)
    sel[0, 0:64] = 1.0
    sel[32, 64:128] = 1.0
    in_maps = []
    for c in range(8):
        b, g = c // 2, c % 2
        wq = w_qkv[g * FG:(g + 1) * FG]
        wk = w_qkv[D + g * FG:D + (g + 1) * FG]
        wv = w_qkv[2 * D + g * FG:2 * D + (g + 1) * FG]
        in_maps.append({
            "xT": np.ascontiguousarray(x[b].T).astype(bf),
            "wqkvT": np.ascontiguousarray(
                np.concatenate([wq.T, wk.T, wv.T], axis=1)).astype(bf),
            "woutT": np.ascontiguousarray(w_out[:, g * FG:(g + 1) * FG].T).astype(bf),
            "mask2": mask2,
            "seld": sel,
        })
    return in_maps


def _ensure_ntff_hook():
    """The agent image's antenv package lacks axon_hooks; shim it so
    run_bass_kernel_spmd(trace=True) can capture NTFF profiles."""
    import sys, types
    try:
        import antenv.axon_hooks  # noqa: F401
        return
    except ImportError:
        pass
    import antenv
    mod = types.ModuleType("antenv.axon_hooks")
    mod._hook = None
    def set_axon_ntff_profile_hook(h):
        mod._hook = h
    def get_axon_ntff_profile_hook():
        return mod._hook
    mod.set_axon_ntff_profile_hook = set_axon_ntff_profile_hook
    mod.get_axon_ntff_profile_hook = get_axon_ntff_profile_hook
    sys.modules["antenv.axon_hooks"] = mod
    antenv.axon_hooks = mod
    try:
        from trn_agent_boot.trn_boot import _ntff_profile_via_ctypes
        set_axon_ntff_profile_hook(
            _ntff_profile_via_ctypes("/opt/axon/libaxon_pjrt.so"))
    except Exception as e:  # degrade to no tracing
        print(f"ntff hook install failed: {e}")


def run(x, w_qkv, w_out, trace=False, trace_kwargs=None, debug=False):
    _import_concourse()
    if trace:
        _ensure_ntff_hook()
    from concourse.bass_utils import run_bass_kernel_spmd

    key = "nc_dbg" if debug else "nc"
    if key not in _CACHE:
        _CACHE[key] = _build_nc(debug_out=debug)
    nc = _CACHE[key]
    in_maps = _make_in_maps(np.asarray(x), np.asarray(w_qkv), np.asarray(w_out))
    kw = dict(trace_kwargs or {})
    res = run_bass_kernel_spmd(nc, in_maps, core_ids=list(range(8)), trace=trace, **kw)
    outs = [r["out"] for r in res.results]
    full = np.empty((B, T, D), dtype=np.float32)
    for b in range(B):
        full[b] = outs[2 * b].astype(np.float32) + outs[2 * b + 1].astype(np.float32)
    return full, res


def kernel(x, w_qkv, w_out):
    full, _ = run(x, w_qkv, w_out, trace=False)
    return full


# revision 52
# speedup vs baseline: 1.0375x; 1.0250x over previous
"""Causal self-attention on 8 TRN2 NeuronCores.

Sharding: core c handles batch b=c//2, head-group g=c%2 (heads g*8..g*8+7).
Each core computes the qkv projection for its 8 heads, causal attention, and
a partial out-projection (its heads' columns of w_out). Host sums the two
partial outputs per batch. All layout transposes are done host-side.

On-chip (per core), P=128 partitions, bf16 matmul operands, f32 PSUM:
  xT    [1024(c), 2048(t)]   x[b] transposed
  wqkvT [1024(c), 1536(f)]   f = [qT 512 | kT 512 | vT 512] for this group
  woutT [512(dv), 1024(o)]   w_out columns for this group, transposed
  qk_sb f-tile m holds q (or k) for heads (2m, 2m+1) at partition offsets
  0/64, so the per-head QK^T matmuls (K=64) run as concurrent PE row-tiles
  T0/T8 (tile_position (0,0)/(64,0)): head pairs stream together, halving
  the score phase.  scoresT[j,i] psum tiles hold the pair side by side
  [A 512 | B 512]; one merged exp (ACT, scale=1/8) skips the not-computed
  sub-diagonal columns, and causal masking is a 0/1 bf16 multiply on the
  exp'd SBUF tile (4x DVE mode) of just the diagonal 128-blocks.
  PV accumulates per head into [65,512] psum via a ones column appended to
  v (row 64 = softmax denominator).  Normalization: in-place
  reciprocal_approx_fast on the psum den row, cast to bf16, then a K=33
  selector matmul broadcasts both heads' 1/den across partitions in one PE
  op; two fused psum-read multiplies write the normalized oT directly.
  Attention runs in 2-j-tile runs with PV lagging one run; QKV for block
  ib+1 and (during ib=3) the out-projection interleave as PE filler.
"""

import math
import numpy as np
import ml_dtypes

B, T, D, H, HD = 4, 2048, 1024, 16, 64
P = 128
HPG = 8          # heads per group
FG = HPG * HD    # 512 features per group
NCC = D // P     # 8 contraction chunks
NTB = 4          # t-blocks of 512
NTT = 16         # t-tiles of 128
NIB = 4          # i-blocks of 512
NPAIR = 4        # head pairs per group
SCALE = 1.0 / math.sqrt(HD)

_CACHE = {}


def _import_concourse():
    """Make concourse importable in environments where it isn't on sys.path."""
    try:
        import concourse.bass  # noqa: F401
        return
    except ImportError:
        pass
    import sys, os
    for p in ("/opt/trn_rl_repo", "/root/.axon_site/_ro/trn_rl_repo"):
        if os.path.isdir(p) and p not in sys.path:
            sys.path.insert(0, p)
    import concourse.bass  # noqa: F401


def _build_nc(debug_out=False):
    _import_concourse()
    from concourse import bacc
    import concourse.mybir as mybir
    import concourse.tile as tile
    from contextlib import ExitStack

    BF = mybir.dt.bfloat16
    F32 = mybir.dt.float32

    nc = bacc.Bacc("TRN2", target_bir_lowering=False, debug=False, num_devices=8)
    xT = nc.dram_tensor("xT", [D, T], BF, kind="ExternalInput").ap()
    wqkvT = nc.dram_tensor("wqkvT", [D, 3 * FG], BF, kind="ExternalInput").ap()
    woutT = nc.dram_tensor("woutT", [FG, D], BF, kind="ExternalInput").ap()
    mask2 = nc.dram_tensor("mask2", [P, 2, P], BF, kind="ExternalInput").ap()
    seld = nc.dram_tensor("seld", [33, P], F32, kind="ExternalInput").ap()
    out = nc.dram_tensor("out", [T, D], BF, kind="ExternalOutput").ap()
    if debug_out:
        qdbg = nc.dram_tensor("qdbg", [P, 8, T], BF, kind="ExternalOutput").ap()
        vdbg = nc.dram_tensor(
            "vdbg", [P, NTT, HPG, HD + 1], BF, kind="ExternalOutput").ap()
        odbg = nc.dram_tensor("odbg", [P, NPAIR, T], BF, kind="ExternalOutput").ap()
        ddbg = nc.dram_tensor(
            "ddbg", [NIB, NPAIR, 33, 512], F32, kind="ExternalOutput").ap()
        bdbg = nc.dram_tensor(
            "bdbg", [NIB, NPAIR, P, 512], BF, kind="ExternalOutput").ap()

    with tile.TileContext(nc) as tc, ExitStack() as ctx:
        singles = ctx.enter_context(tc.tile_pool(name="singles", bufs=1))
        xtp = ctx.enter_context(tc.tile_pool(name="xt", bufs=2))
        ptp = ctx.enter_context(tc.tile_pool(name="pt", bufs=10))
        bcp = ctx.enter_context(tc.tile_pool(name="bc", bufs=3))
        yp = ctx.enter_context(tc.tile_pool(name="y", bufs=3))
        ps_qk = ctx.enter_context(tc.tile_pool(name="ps_qk", bufs=2, space="PSUM"))
        ps_pv = ctx.enter_context(tc.tile_pool(name="ps_pv", bufs=2, space="PSUM"))
        ps_mm = ctx.enter_context(tc.tile_pool(name="ps_mm", bufs=2, space="PSUM"))

        wq_sb = singles.tile([P, NCC, 3 * FG], BF)
        wq_src = wqkvT.rearrange("(cc p) f -> p cc f", p=P)
        # q/k f-tiles land one DMA each in pair-usage order.  The x block-0
        # DMAs are emitted between ft0/ft4 and the rest (inside
        # emit_qkv_block(0) below) so the first matmul group completes
        # ~5us in; the v slab follows for pair 0's PV.
        for ft in (0, 4):
            nc.sync.dma_start(
                out=wq_sb[:, :, ft * P:(ft + 1) * P],
                in_=wq_src[:, :, ft * P:(ft + 1) * P],
            )

        def dma_wq_rest():
            nc.sync.dma_start(
                out=wq_sb[:, :, 2 * FG:3 * FG], in_=wq_src[:, :, 2 * FG:3 * FG]
            )
            for ft in (1, 5, 2, 6, 3, 7):
                nc.sync.dma_start(
                    out=wq_sb[:, :, ft * P:(ft + 1) * P],
                    in_=wq_src[:, :, ft * P:(ft + 1) * P],
                )
        mask_sb = singles.tile([P, 2, P], BF)
        nc.sync.dma_start(out=mask_sb, in_=mask2)
        sel_sb = singles.tile([33, P], F32)
        nc.sync.dma_start(out=sel_sb, in_=seld)
        wo_sb = singles.tile([P, 4, D], BF)

        qk_sb = singles.tile([P, 8, T], BF)              # f-tiles 0..3 = q, 4..7 = k
        vp_sb = singles.tile([P, NTT, HPG, HD + 1], BF)  # [v_h | ones]
        oT_sb = singles.tile([P, NPAIR, T], BF)          # attn out, [dv-pair, t]
        dp = singles.tile([33, 512], F32)                # den rows 0 (A), 32 (B)
        dpr = singles.tile([33, 512], F32)               # 1/den
        nc.vector.memset(vp_sb[:, :, :, HD:HD + 1], 1.0)
        nc.vector.memset(dp, 1.0)

        # ---- filler thunks: qkv blocks + out-projection ----
        def emit_qkv_block(tb):
            """DMA x-block tb, return one thunk per psum group.
            Order: q/k tiles in pair-usage order, then v tiles."""
            xt = xtp.tile([P, NCC, 512], BF)
            xt_src = xT[:, tb * 512:(tb + 1) * 512].rearrange(
                "(cc p) t -> p cc t", p=P)
            # two half-block DMAs: cheap to issue, and cc 0-3 matmuls can
            # start while cc 4-7 are in flight
            nc.sync.dma_start(out=xt[:, 0:4, :], in_=xt_src[:, 0:4, :])
            nc.sync.dma_start(out=xt[:, 4:8, :], in_=xt_src[:, 4:8, :])
            thunks = []
            for ft in range(8):  # q then k feature tiles, output [f=128, t=512]
                def qk_group(ft=ft, xt=xt, tb=tb):  # key ("qk", tb, ft)
                    ps = ps_mm.tile([P, 512], F32)
                    for cc in range(NCC):
                        nc.tensor.matmul(
                            ps,
                            lhsT=wq_sb[:, cc, ft * P:(ft + 1) * P],
                            rhs=xt[:, cc, :],
                            start=(cc == 0),
                            stop=(cc == NCC - 1),
                        )
                    nc.vector.tensor_copy(
                        out=qk_sb[:, ft, tb * 512:(tb + 1) * 512], in_=ps
                    )
                thunks.append((("qk", tb, ft), qk_group))
            for tl in range(4):  # v in [t, dv] orientation, output [t=128, dv=512]
                def v_group(tl=tl, xt=xt, tb=tb):
                    tt = tb * 4 + tl
                    ps = ps_mm.tile([P, FG], F32)
                    for cc in range(NCC):
                        nc.tensor.matmul(
                            ps,
                            lhsT=xt[:, cc, tl * P:(tl + 1) * P],
                            rhs=wq_sb[:, cc, 2 * FG:3 * FG],
                            start=(cc == 0),
                            stop=(cc == NCC - 1),
                        )
                    nc.vector.tensor_copy(
                        out=vp_sb[:, tt, :, 0:HD],
                        in_=ps.rearrange("p (h d) -> p h d", h=HPG),
                    )
                thunks.append((("v", tb * 4 + tl), v_group))
            # interleave q/k pair-wise: q0,k0,q1,k1,... then v0..v3
            order = [0, 4, 1, 5, 2, 6, 3, 7, 8, 9, 10, 11]
            return [thunks[i] for i in order]

        def emit_outproj_tt(tt, tail=False):
            yt = yp.tile([P, 1024], BF)
            for ob in range(2):
                ps = ps_mm.tile([P, 512], F32, tag="ps", name="ps_op")
                for dc in range(4):
                    nc.tensor.matmul(
                        ps,
                        lhsT=oT_sb[:, dc, tt * P:(tt + 1) * P],
                        rhs=wo_sb[:, dc, ob * 512:(ob + 1) * 512],
                        start=(dc == 0),
                        stop=(dc == 3),
                    )
                if tail:  # ACT is idle in the tail; halve the copy chain
                    nc.scalar.copy(out=yt[:, ob * 512:(ob + 1) * 512], in_=ps)
                else:
                    nc.vector.tensor_copy(yt[:, ob * 512:(ob + 1) * 512], ps)
            nc.sync.dma_start(out=out[tt * P:(tt + 1) * P, :], in_=yt)

        # ---- attention ----
        def emit_attn_pair(ib, m, pop_filler, need, finish_prev):
            """Scores+softmax+PV for head pair (2m, 2m+1), i-block ib.
            pop_filler() emits paced filler; need(keys) force-drains filler
            thunks this pair reads from; finish_prev finishes the previous
            pair's normalization (emitted inside the 64-mode QK runs)."""
            njt = 4 * ib + 4
            isl = slice(ib * 512, (ib + 1) * 512)
            fq, fk = m, 4 + m
            pvA = ps_pv.tile([HD + 1, 512], F32, tag="pv", name="pvA")
            pvB = ps_pv.tile([HD + 1, 512], F32, tag="pv", name="pvB")
            pts = {}

            def qk_run(jts, extra=None):
                for jt in jts:
                    r = jt - 4 * ib
                    c0 = P * r if r > 0 else 0
                    ps = ps_qk.tile([P, 1024], F32)
                    nc.tensor.matmul(
                        ps[:, c0:512],
                        lhsT=qk_sb[0:64, fk, jt * P:(jt + 1) * P],
                        rhs=qk_sb[0:64, fq, ib * 512 + c0:(ib + 1) * 512],
                        start=True, stop=True,
                    )
                    nc.tensor.matmul(
                        ps[:, 512 + c0:1024],
                        lhsT=qk_sb[64:128, fk, jt * P:(jt + 1) * P],
                        rhs=qk_sb[64:128, fq, ib * 512 + c0:(ib + 1) * 512],
                        start=True, stop=True,
                    )
                    if extra is not None:  # same-mode (64,128) deferred work
                        extra(); extra = None
                    pt = ptp.tile([P, 1024], BF)
                    ps2 = ps.rearrange("p (g w) -> p g w", g=2)
                    pt2 = pt.rearrange("p (g w) -> p g w", g=2)
                    nc.scalar.activation(
                        out=pt2[:, :, c0:512], in_=ps2[:, :, c0:512],
                        func=mybir.ActivationFunctionType.Exp, scale=SCALE,
                    )
                    if r >= 0:  # zero the upper triangle of the diagonal block
                        nc.vector.tensor_mul(
                            pt2[:, :, c0:c0 + P], pt2[:, :, c0:c0 + P], mask_sb
                        )
                    pts[jt] = (pt, c0)
                if extra is not None:
                    extra()

            def pv_run(jts):
                for jt in jts:
                    pt, c0 = pts.pop(jt)
                    pt2 = pt.rearrange("p (g w) -> p g w", g=2)
                    first, last = (jt == 0), (jt == njt - 1)
                    nc.tensor.matmul(
                        pvA[:, c0:512],
                        lhsT=vp_sb[:, jt, 2 * m, :],
                        rhs=pt2[:, 0, c0:512],
                        start=first, stop=last, skip_group_check=True,
                    )
                    nc.tensor.matmul(
                        pvB[:, c0:512],
                        lhsT=vp_sb[:, jt, 2 * m + 1, :],
                        rhs=pt2[:, 1, c0:512],
                        start=first, stop=last, skip_group_check=True,
                    )

            need([("qk", tb, fk) for tb in range(ib + 1)] + [("qk", ib, fq)])
            # longer runs in the ACT-bound later blocks: fewer PE mode
            # switches, and the QK->exp chain paces itself there anyway
            rsz = 2 if ib < 2 else 4
            runs = [list(range(s, min(s + rsz, njt)))
                    for s in range(0, njt, rsz)]
            prev = None
            for ri, run in enumerate(runs):
                qk_run(run, extra=(finish_prev if ri == min(1, len(runs) - 1)
                                   else None))
                pop_filler()  # filler before PV: exps + normalize get slack
                if ri == 0:  # extra filler at the pair boundary, where the
                    pop_filler()  # previous pair's normalization needs slack
                if prev is not None:
                    need([("v", jt) for jt in prev])
                    pv_run(prev)
                prev = run
            need([("v", jt) for jt in prev])
            pv_run(prev)

            # start normalization: gather dens to SBUF, 1/den, cast to bf16
            nc.vector.tensor_copy(out=dp[0:1, :], in_=pvA[HD:HD + 1, :])
            nc.vector.tensor_copy(out=dp[32:33, :], in_=pvB[HD:HD + 1, :])
            nc.vector.reciprocal_approx_fast(out=dpr, in_=dp)
            if debug_out:
                nc.sync.dma_start(out=ddbg[ib, m], in_=dpr)

            def finish():
                # broadcast 1/den across partitions with a K=33 selector
                # matmul (64-mode, emitted inside the next pair's QK run),
                # then two fused psum-read muls write normalized oT.
                bc_ps = ps_mm.tile([P, 512], F32, tag="ps", name="bc_ps")
                nc.tensor.matmul(bc_ps, lhsT=sel_sb, rhs=dpr,
                                 start=True, stop=True)
                bc = bcp.tile([P, 512], BF)
                nc.vector.tensor_copy(out=bc, in_=bc_ps)
                if debug_out:
                    nc.sync.dma_start(out=bdbg[ib, m], in_=bc)
                nc.vector.tensor_mul(
                    oT_sb[0:HD, m, isl], pvA[0:HD, :], bc[0:64, :])
                nc.vector.tensor_mul(
                    oT_sb[64:64 + HD, m, isl], pvB[0:HD, :], bc[64:128, :])
            return finish

        # ---- top-level emission ----
        filler = []          # list of (key, fn)
        fill_done = 0
        emitted = set()

        def drain_one():
            nonlocal fill_done
            key, fn = filler[fill_done]
            fn()
            emitted.add(key)
            fill_done += 1

        def need(keys):
            while not all(k in emitted for k in keys):
                drain_one()

        def make_pop(total_runs):
            state = {"run": 0, "base": fill_done}
            def pop():
                state["run"] += 1
                want = min(len(filler), state["base"] + (
                    (len(filler) - state["base"]) * state["run"] // total_runs))
                while fill_done < want:
                    drain_one()
            return pop

        filler += emit_qkv_block(0)   # emits the x block-0 DMAs
        dma_wq_rest()
        # pull the first q/k pair groups upfront so attention starts early
        drain_one(); drain_one()

        finish_prev = None
        for ib in range(NIB):
            if ib == 1:  # wo is needed from ib3; keep it off the hot window
                nc.sync.dma_start(
                    out=wo_sb, in_=woutT.rearrange("(dc p) o -> p dc o", p=P))
            if ib < 3:
                filler.extend(emit_qkv_block(ib + 1))
            else:
                # tts 0-7 read oT of ib0/ib1, fully normalized by now; tts
                # 8-11 (ib2 rows) need finish(ib2, pair3), emitted during
                # pair 0 below, so they join the filler list after it.
                filler.extend(
                    ((("op", tt), (lambda tt=tt: emit_outproj_tt(tt)))
                     for tt in range(8)))
            rsz_ib = 2 if ib < 2 else 4
            npops = -(-(4 * ib + 4) // rsz_ib) + 1  # pops per pair
            pop = make_pop(total_runs=NPAIR * npops)
            for m in range(NPAIR):
                finish_prev = emit_attn_pair(ib, m, pop, need, finish_prev)
                if ib == 3 and m == 0:
                    filler.extend(
                        ((("op", tt), (lambda tt=tt: emit_outproj_tt(tt)))
                         for tt in range(8, 12)))
        finish_prev()
        while fill_done < len(filler):
            drain_one()
        for tt in range(12, 16):
            emit_outproj_tt(tt, tail=True)
        if debug_out:
            nc.sync.dma_start(out=qdbg, in_=qk_sb)
            nc.sync.dma_start(out=vdbg, in_=vp_sb)
            nc.sync.dma_start(out=odbg, in_=oT_sb)
    nc.compile()
    return nc


def _make_in_maps(x, w_qkv, w_out):
    bf = ml_dtypes.bfloat16
    # 0/1 mask for the diagonal 128x128 block: keep i_local >= j_local
    tri = (np.arange(P)[None, :] >= np.arange(P)[:, None]).astype(np.float32)
    mask2 = np.ascontiguousarray(
        np.broadcast_to(tri[:, None, :], (P, 2, P))).astype(bf)
    sel = np.zeros((33, P), dtype=np.float32)
    sel[0, 0:64] = 1.0
    sel[32, 64:128] = 1.0
    in_maps = []
    for c in range(8):
        b, g = c // 2, c % 2
        wq = w_qkv[g * FG:(g + 1) * FG]
        wk = w_qkv[D + g * FG:D + (g + 1) * FG]
        wv = w_qkv[2 * D + g * FG:2 * D + (g + 1) * FG]
        in_maps.append({
            "xT": np.ascontiguousarray(x[b].T).astype(bf),
            "wqkvT": np.ascontiguousarray(
                np.concatenate([wq.T, wk.T, wv.T], axis=1)).astype(bf),
            "woutT": np.ascontiguousarray(w_out[:, g * FG:(g + 1) * FG].T).astype(bf),
            "mask2": mask2,
            "seld": sel,
        })
    return in_maps


def _ensure_ntff_hook():
    """The agent image's antenv package lacks axon_hooks; shim it so
    run_bass_kernel_spmd(trace=True) can capture NTFF profiles."""
    import sys, types
    try:
        import antenv.axon_hooks  # noqa: F401
        return
    except ImportError:
        pass
    import antenv
    mod = types.ModuleType("antenv.axon_hooks")
    mod._hook = None
    def set_axon_ntff_profile_hook(h):
        mod._hook = h
    def get_axon_ntff_profile_hook():
        return mod._hook
    mod.set_axon_ntff_profile_hook = set_axon_ntff_profile_hook
    mod.get_axon_ntff_profile_hook = get_axon_ntff_profile_hook
    sys.modules["antenv.axon_hooks"] = mod
    antenv.axon_hooks = mod
    try:
        from trn_agent_boot.trn_boot import _ntff_profile_via_ctypes
        set_axon_ntff_profile_hook(
            _ntff_profile_via_ctypes("/opt/axon/libaxon_pjrt.so"))
    except Exception as e:  # degrade to no tracing
        print(f"ntff hook install failed: {e}")


def run(x, w_qkv, w_out, trace=False, trace_kwargs=None, debug=False):
    _import_concourse()
    if trace:
        _ensure_ntff_hook()
    from concourse.bass_utils import run_bass_kernel_spmd

    key = "nc_dbg" if debug else "nc"
    if key not in _CACHE:
        _CACHE[key] = _build_nc(debug_out=debug)
    nc = _CACHE[key]
    in_maps = _make_in_maps(np.asarray(x), np.asarray(w_qkv), np.asarray(w_out))
    kw = dict(trace_kwargs or {})
    res = run_bass_kernel_spmd(nc, in_maps, core_ids=list(range(8)), trace=trace, **kw)
    outs = [r["out"] for r in res.results]
    full = np.empty((B, T, D), dtype=np.float32)
    for b in range(B):
        full[b] = outs[2 * b].astype(np.float32) + outs[2 * b + 1].astype(np.float32)
    return full, res


def kernel(x, w_qkv, w_out):
    full, _ = run(x, w_qkv, w_out, trace=False)
    return full


# revision 58
# speedup vs baseline: 1.0430x; 1.0053x over previous
"""Causal self-attention on 8 TRN2 NeuronCores.

Sharding: core c handles batch b=c//2, head-group g=c%2 (heads g*8..g*8+7).
Each core computes the qkv projection for its 8 heads, causal attention, and
a partial out-projection (its heads' columns of w_out). Host sums the two
partial outputs per batch. All layout transposes are done host-side.

On-chip (per core), P=128 partitions, bf16 matmul operands, f32 PSUM:
  xT    [1024(c), 2048(t)]   x[b] transposed
  wqkvT [1024(c), 1536(f)]   f = [qT 512 | kT 512 | vT 512] for this group
  woutT [512(dv), 1024(o)]   w_out columns for this group, transposed
  qk_sb f-tile m holds q (or k) for heads (2m, 2m+1) at partition offsets
  0/64, so the per-head QK^T matmuls (K=64) run as concurrent PE row-tiles
  T0/T8 (tile_position (0,0)/(64,0)): head pairs stream together, halving
  the score phase.  scoresT[j,i] psum tiles hold the pair side by side
  [A 512 | B 512]; one merged exp (ACT, scale=1/8) skips the not-computed
  sub-diagonal columns, and causal masking is a 0/1 bf16 multiply on the
  exp'd SBUF tile (4x DVE mode) of just the diagonal 128-blocks.
  PV accumulates per head into [65,512] psum via a ones column appended to
  v (row 64 = softmax denominator).  Normalization: in-place
  reciprocal_approx_fast on the psum den row, cast to bf16, then a K=33
  selector matmul broadcasts both heads' 1/den across partitions in one PE
  op; two fused psum-read multiplies write the normalized oT directly.
  Attention runs in 2-j-tile runs with PV lagging one run; QKV for block
  ib+1 and (during ib=3) the out-projection interleave as PE filler.
"""

import math
import numpy as np
import ml_dtypes

B, T, D, H, HD = 4, 2048, 1024, 16, 64
P = 128
HPG = 8          # heads per group
FG = HPG * HD    # 512 features per group
NCC = D // P     # 8 contraction chunks
NTB = 4          # t-blocks of 512
NTT = 16         # t-tiles of 128
NIB = 4          # i-blocks of 512
NPAIR = 4        # head pairs per group
SCALE = 1.0 / math.sqrt(HD)

_CACHE = {}


def _import_concourse():
    """Make concourse importable in environments where it isn't on sys.path."""
    try:
        import concourse.bass  # noqa: F401
        return
    except ImportError:
        pass
    import sys, os
    for p in ("/opt/trn_rl_repo", "/root/.axon_site/_ro/trn_rl_repo"):
        if os.path.isdir(p) and p not in sys.path:
            sys.path.insert(0, p)
    import concourse.bass  # noqa: F401


def _build_nc(debug_out=False):
    _import_concourse()
    from concourse import bacc
    import concourse.mybir as mybir
    import concourse.tile as tile
    from contextlib import ExitStack

    BF = mybir.dt.bfloat16
    F32 = mybir.dt.float32

    nc = bacc.Bacc("TRN2", target_bir_lowering=False, debug=False, num_devices=8)
    xT = nc.dram_tensor("xT", [D, T], BF, kind="ExternalInput").ap()
    wqkvT = nc.dram_tensor("wqkvT", [D, 3 * FG], BF, kind="ExternalInput").ap()
    woutT = nc.dram_tensor("woutT", [FG, D], BF, kind="ExternalInput").ap()
    mask2 = nc.dram_tensor("mask2", [P, 2, P], BF, kind="ExternalInput").ap()
    seld = nc.dram_tensor("seld", [33, P], F32, kind="ExternalInput").ap()
    out = nc.dram_tensor("out", [T, D], BF, kind="ExternalOutput").ap()
    if debug_out:
        qdbg = nc.dram_tensor("qdbg", [P, 8, T], BF, kind="ExternalOutput").ap()
        vdbg = nc.dram_tensor(
            "vdbg", [P, NTT, HPG, HD + 1], BF, kind="ExternalOutput").ap()
        odbg = nc.dram_tensor("odbg", [P, NPAIR, T], BF, kind="ExternalOutput").ap()
        ddbg = nc.dram_tensor(
            "ddbg", [NIB, NPAIR, 33, 512], F32, kind="ExternalOutput").ap()
        bdbg = nc.dram_tensor(
            "bdbg", [NIB, NPAIR, P, 512], BF, kind="ExternalOutput").ap()

    with tile.TileContext(nc) as tc, ExitStack() as ctx:
        singles = ctx.enter_context(tc.tile_pool(name="singles", bufs=1))
        xtp = ctx.enter_context(tc.tile_pool(name="xt", bufs=2))
        ptp = ctx.enter_context(tc.tile_pool(name="pt", bufs=10))
        bcp = ctx.enter_context(tc.tile_pool(name="bc", bufs=3))
        yp = ctx.enter_context(tc.tile_pool(name="y", bufs=3))
        ps_qk = ctx.enter_context(tc.tile_pool(name="ps_qk", bufs=2, space="PSUM"))
        ps_pv = ctx.enter_context(tc.tile_pool(name="ps_pv", bufs=2, space="PSUM"))
        ps_mm = ctx.enter_context(tc.tile_pool(name="ps_mm", bufs=2, space="PSUM"))

        wq_sb = singles.tile([P, NCC, 3 * FG], BF)
        wq_src = wqkvT.rearrange("(cc p) f -> p cc f", p=P)
        # q/k f-tiles land one DMA each in pair-usage order.  The x block-0
        # DMAs are emitted between ft0/ft4 and the rest (inside
        # emit_qkv_block(0) below) so the first matmul group completes
        # ~5us in; the v slab follows for pair 0's PV.
        for ft in (0, 4):
            nc.sync.dma_start(
                out=wq_sb[:, :, ft * P:(ft + 1) * P],
                in_=wq_src[:, :, ft * P:(ft + 1) * P],
            )

        def dma_wq_rest():
            nc.sync.dma_start(
                out=wq_sb[:, :, 2 * FG:3 * FG], in_=wq_src[:, :, 2 * FG:3 * FG]
            )
            for ft in (1, 5, 2, 6, 3, 7):
                nc.sync.dma_start(
                    out=wq_sb[:, :, ft * P:(ft + 1) * P],
                    in_=wq_src[:, :, ft * P:(ft + 1) * P],
                )
        mask_sb = singles.tile([P, 2, P], BF)
        nc.sync.dma_start(out=mask_sb, in_=mask2)
        sel_sb = singles.tile([33, P], F32)
        nc.sync.dma_start(out=sel_sb, in_=seld)
        wo_sb = singles.tile([P, 4, D], BF)

        qk_sb = singles.tile([P, 8, T], BF)              # f-tiles 0..3 = q, 4..7 = k
        vp_sb = singles.tile([P, NTT, HPG, HD + 1], BF)  # [v_h | ones]
        oT_sb = singles.tile([P, NPAIR, T], BF)          # attn out, [dv-pair, t]
        dp = singles.tile([33, 512], F32)                # den rows 0 (A), 32 (B)
        dpr = singles.tile([33, 512], F32)               # 1/den
        nc.vector.memset(vp_sb[:, :, :, HD:HD + 1], 1.0)
        nc.vector.memset(dp, 1.0)

        # ---- filler thunks: qkv blocks + out-projection ----
        def emit_qkv_block(tb):
            """DMA x-block tb, return one thunk per psum group.
            Order: q/k tiles in pair-usage order, then v tiles."""
            xt = xtp.tile([P, NCC, 512], BF)
            xt_src = xT[:, tb * 512:(tb + 1) * 512].rearrange(
                "(cc p) t -> p cc t", p=P)
            # two half-block DMAs: cheap to issue, and cc 0-3 matmuls can
            # start while cc 4-7 are in flight
            nc.sync.dma_start(out=xt[:, 0:4, :], in_=xt_src[:, 0:4, :])
            nc.sync.dma_start(out=xt[:, 4:8, :], in_=xt_src[:, 4:8, :])
            thunks = []
            for ft in range(8):  # q then k feature tiles, output [f=128, t=512]
                def qk_group(ft=ft, xt=xt, tb=tb):  # key ("qk", tb, ft)
                    ps = ps_mm.tile([P, 512], F32)
                    for cc in range(NCC):
                        nc.tensor.matmul(
                            ps,
                            lhsT=wq_sb[:, cc, ft * P:(ft + 1) * P],
                            rhs=xt[:, cc, :],
                            start=(cc == 0),
                            stop=(cc == NCC - 1),
                        )
                    nc.vector.tensor_copy(
                        out=qk_sb[:, ft, tb * 512:(tb + 1) * 512], in_=ps
                    )
                thunks.append((("qk", tb, ft), qk_group))
            for tl in range(4):  # v in [t, dv] orientation, output [t=128, dv=512]
                def v_group(tl=tl, xt=xt, tb=tb):
                    tt = tb * 4 + tl
                    ps = ps_mm.tile([P, FG], F32)
                    for cc in range(NCC):
                        nc.tensor.matmul(
                            ps,
                            lhsT=xt[:, cc, tl * P:(tl + 1) * P],
                            rhs=wq_sb[:, cc, 2 * FG:3 * FG],
                            start=(cc == 0),
                            stop=(cc == NCC - 1),
                        )
                    nc.vector.tensor_copy(
                        out=vp_sb[:, tt, :, 0:HD],
                        in_=ps.rearrange("p (h d) -> p h d", h=HPG),
                    )
                thunks.append((("v", tb * 4 + tl), v_group))
            # interleave q/k pair-wise: q0,k0,q1,k1,... then v0..v3
            order = [0, 4, 1, 5, 2, 6, 3, 7, 8, 9, 10, 11]
            return [thunks[i] for i in order]

        def emit_outproj_tt(tt, tail=False):
            yt = yp.tile([P, 1024], BF)
            for ob in range(2):
                ps = ps_mm.tile([P, 512], F32, tag="ps", name="ps_op")
                for dc in range(4):
                    nc.tensor.matmul(
                        ps,
                        lhsT=oT_sb[:, dc, tt * P:(tt + 1) * P],
                        rhs=wo_sb[:, dc, ob * 512:(ob + 1) * 512],
                        start=(dc == 0),
                        stop=(dc == 3),
                    )
                if tail:  # ACT is idle in the tail; halve the copy chain
                    nc.scalar.copy(out=yt[:, ob * 512:(ob + 1) * 512], in_=ps)
                else:
                    nc.vector.tensor_copy(yt[:, ob * 512:(ob + 1) * 512], ps)
            nc.sync.dma_start(out=out[tt * P:(tt + 1) * P, :], in_=yt)

        # ---- attention ----
        def emit_attn_pair(ib, m, pop_filler, need, finish_prev):
            """Scores+softmax+PV for head pair (2m, 2m+1), i-block ib.
            pop_filler() emits paced filler; need(keys) force-drains filler
            thunks this pair reads from; finish_prev finishes the previous
            pair's normalization (emitted inside the 64-mode QK runs)."""
            njt = 4 * ib + 4
            isl = slice(ib * 512, (ib + 1) * 512)
            fq, fk = m, 4 + m
            pvA = ps_pv.tile([HD + 1, 512], F32, tag="pv", name="pvA")
            pvB = ps_pv.tile([HD + 1, 512], F32, tag="pv", name="pvB")
            pts = {}

            def qk_run(jts, extra=None):
                for jt in jts:
                    r = jt - 4 * ib
                    c0 = P * r if r > 0 else 0
                    ps = ps_qk.tile([P, 1024], F32)
                    nc.tensor.matmul(
                        ps[:, c0:512],
                        lhsT=qk_sb[0:64, fk, jt * P:(jt + 1) * P],
                        rhs=qk_sb[0:64, fq, ib * 512 + c0:(ib + 1) * 512],
                        start=True, stop=True,
                    )
                    nc.tensor.matmul(
                        ps[:, 512 + c0:1024],
                        lhsT=qk_sb[64:128, fk, jt * P:(jt + 1) * P],
                        rhs=qk_sb[64:128, fq, ib * 512 + c0:(ib + 1) * 512],
                        start=True, stop=True,
                    )
                    if extra is not None:  # same-mode (64,128) deferred work
                        extra(); extra = None
                    pt = ptp.tile([P, 1024], BF)
                    ps2 = ps.rearrange("p (g w) -> p g w", g=2)
                    pt2 = pt.rearrange("p (g w) -> p g w", g=2)
                    nc.scalar.activation(
                        out=pt2[:, :, c0:512], in_=ps2[:, :, c0:512],
                        func=mybir.ActivationFunctionType.Exp, scale=SCALE,
                    )
                    if r >= 0:  # zero the upper triangle of the diagonal block
                        nc.vector.tensor_mul(
                            pt2[:, :, c0:c0 + P], pt2[:, :, c0:c0 + P], mask_sb
                        )
                    pts[jt] = (pt, c0)
                if extra is not None:
                    extra()

            def pv_run(jts):
                for jt in jts:
                    pt, c0 = pts.pop(jt)
                    pt2 = pt.rearrange("p (g w) -> p g w", g=2)
                    first, last = (jt == 0), (jt == njt - 1)
                    nc.tensor.matmul(
                        pvA[:, c0:512],
                        lhsT=vp_sb[:, jt, 2 * m, :],
                        rhs=pt2[:, 0, c0:512],
                        start=first, stop=last, skip_group_check=True,
                    )
                    nc.tensor.matmul(
                        pvB[:, c0:512],
                        lhsT=vp_sb[:, jt, 2 * m + 1, :],
                        rhs=pt2[:, 1, c0:512],
                        start=first, stop=last, skip_group_check=True,
                    )

            need([("qk", tb, fk) for tb in range(ib + 1)] + [("qk", ib, fq)])
            # longer runs in the ACT-bound later blocks: fewer PE mode
            # switches, and the QK->exp chain paces itself there anyway
            rsz = 2 if ib < 2 else 4
            runs = [list(range(s, min(s + rsz, njt)))
                    for s in range(0, njt, rsz)]
            prev = None
            for ri, run in enumerate(runs):
                qk_run(run, extra=(finish_prev if ri == min(1, len(runs) - 1)
                                   else None))
                pop_filler()  # filler before PV: exps + normalize get slack
                if ri <= 1:  # extra filler at the pair boundary, where the
                    pop_filler()  # previous pair's normalization needs slack
                if prev is not None:
                    need([("v", jt) for jt in prev])
                    pv_run(prev)
                prev = run
            need([("v", jt) for jt in prev])
            pv_run(prev)

            # start normalization: gather dens to SBUF (custom DVE ops
            # cannot touch PSUM), then one batched 1/den
            nc.vector.tensor_copy(out=dp[0:1, :], in_=pvA[HD:HD + 1, :])
            nc.vector.tensor_copy(out=dp[32:33, :], in_=pvB[HD:HD + 1, :])
            nc.vector.reciprocal_approx_fast(out=dpr, in_=dp)
            if debug_out:
                nc.sync.dma_start(out=ddbg[ib, m], in_=dpr)

            def finish():
                # broadcast 1/den across partitions with a K=33 selector
                # matmul (64-mode, emitted inside the next pair's QK run),
                # then two fused psum-read muls write normalized oT.
                bc_ps = ps_mm.tile([P, 512], F32, tag="ps", name="bc_ps")
                nc.tensor.matmul(bc_ps, lhsT=sel_sb, rhs=dpr,
                                 start=True, stop=True)
                bc = bcp.tile([P, 512], BF)
                nc.vector.tensor_copy(out=bc, in_=bc_ps)
                if debug_out:
                    nc.sync.dma_start(out=bdbg[ib, m], in_=bc)
                nc.vector.tensor_mul(
                    oT_sb[0:HD, m, isl], pvA[0:HD, :], bc[0:64, :])
                nc.vector.tensor_mul(
                    oT_sb[64:64 + HD, m, isl], pvB[0:HD, :], bc[64:128, :])
            return finish

        # ---- top-level emission ----
        filler = []          # list of (key, fn)
        fill_done = 0
        emitted = set()

        def drain_one():
            nonlocal fill_done
            key, fn = filler[fill_done]
            fn()
            emitted.add(key)
            fill_done += 1

        def need(keys):
            while not all(k in emitted for k in keys):
                drain_one()

        def make_pop(total_runs):
            state = {"run": 0, "base": fill_done}
            def pop():
                state["run"] += 1
                want = min(len(filler), state["base"] + (
                    (len(filler) - state["base"]) * state["run"] // total_runs))
                while fill_done < want:
                    drain_one()
            return pop

        filler += emit_qkv_block(0)   # emits the x block-0 DMAs
        dma_wq_rest()
        # pull the first q/k pair groups upfront so attention starts early
        drain_one(); drain_one()

        finish_prev = None
        for ib in range(NIB):
            if ib == 1:  # wo is needed from ib3; keep it off the hot window
                nc.sync.dma_start(
                    out=wo_sb, in_=woutT.rearrange("(dc p) o -> p dc o", p=P))
            if ib < 3:
                filler.extend(emit_qkv_block(ib + 1))
            else:
                # tts 0-7 read oT of ib0/ib1, fully normalized by now; tts
                # 8-11 (ib2 rows) need finish(ib2, pair3), emitted during
                # pair 0 below, so they join the filler list after it.
                filler.extend(
                    ((("op", tt), (lambda tt=tt: emit_outproj_tt(tt)))
                     for tt in range(8)))
            rsz_ib = 2 if ib < 2 else 4
            npops = -(-(4 * ib + 4) // rsz_ib) + 2  # pops per pair
            pop = make_pop(total_runs=NPAIR * npops)
            for m in range(NPAIR):
                finish_prev = emit_attn_pair(ib, m, pop, need, finish_prev)
                if ib == 3 and m == 0:
                    filler.extend(
                        ((("op", tt), (lambda tt=tt: emit_outproj_tt(tt)))
                         for tt in range(8, 12)))
        finish_prev()
        while fill_done < len(filler):
            drain_one()
        for tt in range(12, 16):
            emit_outproj_tt(tt, tail=True)
        if debug_out:
            nc.sync.dma_start(out=qdbg, in_=qk_sb)
            nc.sync.dma_start(out=vdbg, in_=vp_sb)
            nc.sync.dma_start(out=odbg, in_=oT_sb)
    nc.compile()
    return nc


def _make_in_maps(x, w_qkv, w_out):
    bf = ml_dtypes.bfloat16
    # 0/1 mask for the diagonal 128x128 block: keep i_local >= j_local
    tri = (np.arange(P)[None, :] >= np.arange(P)[:, None]).astype(np.float32)
    mask2 = np.ascontiguousarray(
        np.broadcast_to(tri[:, None, :], (P, 2, P))).astype(bf)
    sel = np.zeros((33, P), dtype=np.float32)
    sel[0, 0:64] = 1.0
    sel[32, 64:128] = 1.0
    in_maps = []
    for c in range(8):
        b, g = c // 2, c % 2
        wq = w_qkv[g * FG:(g + 1) * FG]
        wk = w_qkv[D + g * FG:D + (g + 1) * FG]
        wv = w_qkv[2 * D + g * FG:2 * D + (g + 1) * FG]
        in_maps.append({
            "xT": np.ascontiguousarray(x[b].T).astype(bf),
            "wqkvT": np.ascontiguousarray(
                np.concatenate([wq.T, wk.T, wv.T], axis=1)).astype(bf),
            "woutT": np.ascontiguousarray(w_out[:, g * FG:(g + 1) * FG].T).astype(bf),
            "mask2": mask2,
            "seld": sel,
        })
    return in_maps


def _ensure_ntff_hook():
    """The agent image's antenv package lacks axon_hooks; shim it so
    run_bass_kernel_spmd(trace=True) can capture NTFF profiles."""
    import sys, types
    try:
        import antenv.axon_hooks  # noqa: F401
        return
    except ImportError:
        pass
    import antenv
    mod = types.ModuleType("antenv.axon_hooks")
    mod._hook = None
    def set_axon_ntff_profile_hook(h):
        mod._hook = h
    def get_axon_ntff_profile_hook():
        return mod._hook
    mod.set_axon_ntff_profile_hook = set_axon_ntff_profile_hook
    mod.get_axon_ntff_profile_hook = get_axon_ntff_profile_hook
    sys.modules["antenv.axon_hooks"] = mod
    antenv.axon_hooks = mod
    try:
        from trn_agent_boot.trn_boot import _ntff_profile_via_ctypes
        set_axon_ntff_profile_hook(
            _ntff_profile_via_ctypes("/opt/axon/libaxon_pjrt.so"))
    except Exception as e:  # degrade to no tracing
        print(f"ntff hook install failed: {e}")


def run(x, w_qkv, w_out, trace=False, trace_kwargs=None, debug=False):
    _import_concourse()
    if trace:
        _ensure_ntff_hook()
    from concourse.bass_utils import run_bass_kernel_spmd

    key = "nc_dbg" if debug else "nc"
    if key not in _CACHE:
        _CACHE[key] = _build_nc(debug_out=debug)
    nc = _CACHE[key]
    in_maps = _make_in_maps(np.asarray(x), np.asarray(w_qkv), np.asarray(w_out))
    kw = dict(trace_kwargs or {})
    res = run_bass_kernel_spmd(nc, in_maps, core_ids=list(range(8)), trace=trace, **kw)
    outs = [r["out"] for r in res.results]
    full = np.empty((B, T, D), dtype=np.float32)
    for b in range(B):
        full[b] = outs[2 * b].astype(np.float32) + outs[2 * b + 1].astype(np.float32)
    return full, res


def kernel(x, w_qkv, w_out):
    full, _ = run(x, w_qkv, w_out, trace=False)
    return full


# revision 61
# speedup vs baseline: 1.0484x; 1.0051x over previous
"""Causal self-attention on 8 TRN2 NeuronCores.

Sharding: core c handles batch b=c//2, head-group g=c%2 (heads g*8..g*8+7).
Each core computes the qkv projection for its 8 heads, causal attention, and
a partial out-projection (its heads' columns of w_out). Host sums the two
partial outputs per batch. All layout transposes are done host-side.

On-chip (per core), P=128 partitions, bf16 matmul operands, f32 PSUM:
  xT    [1024(c), 2048(t)]   x[b] transposed
  wqkvT [1024(c), 1536(f)]   f = [qT 512 | kT 512 | vT 512] for this group
  woutT [512(dv), 1024(o)]   w_out columns for this group, transposed
  qk_sb f-tile m holds q (or k) for heads (2m, 2m+1) at partition offsets
  0/64, so the per-head QK^T matmuls (K=64) run as concurrent PE row-tiles
  T0/T8 (tile_position (0,0)/(64,0)): head pairs stream together, halving
  the score phase.  scoresT[j,i] psum tiles hold the pair side by side
  [A 512 | B 512]; one merged exp (ACT, scale=1/8) skips the not-computed
  sub-diagonal columns, and causal masking is a 0/1 bf16 multiply on the
  exp'd SBUF tile (4x DVE mode) of just the diagonal 128-blocks.
  PV accumulates per head into [65,512] psum via a ones column appended to
  v (row 64 = softmax denominator).  Normalization: in-place
  reciprocal_approx_fast on the psum den row, cast to bf16, then a K=33
  selector matmul broadcasts both heads' 1/den across partitions in one PE
  op; two fused psum-read multiplies write the normalized oT directly.
  Attention runs in 2-j-tile runs with PV lagging one run; QKV for block
  ib+1 and (during ib=3) the out-projection interleave as PE filler.
"""

import math
import numpy as np
import ml_dtypes

B, T, D, H, HD = 4, 2048, 1024, 16, 64
P = 128
HPG = 8          # heads per group
FG = HPG * HD    # 512 features per group
NCC = D // P     # 8 contraction chunks
NTB = 4          # t-blocks of 512
NTT = 16         # t-tiles of 128
NIB = 4          # i-blocks of 512
NPAIR = 4        # head pairs per group
SCALE = 1.0 / math.sqrt(HD)

_CACHE = {}


def _import_concourse():
    """Make concourse importable in environments where it isn't on sys.path."""
    try:
        import concourse.bass  # noqa: F401
        return
    except ImportError:
        pass
    import sys, os
    for p in ("/opt/trn_rl_repo", "/root/.axon_site/_ro/trn_rl_repo"):
        if os.path.isdir(p) and p not in sys.path:
            sys.path.insert(0, p)
    import concourse.bass  # noqa: F401


def _build_nc(debug_out=False):
    _import_concourse()
    from concourse import bacc
    import concourse.mybir as mybir
    import concourse.tile as tile
    from contextlib import ExitStack

    BF = mybir.dt.bfloat16
    F32 = mybir.dt.float32

    nc = bacc.Bacc("TRN2", target_bir_lowering=False, debug=False, num_devices=8)
    xT = nc.dram_tensor("xT", [D, T], BF, kind="ExternalInput").ap()
    wqkvT = nc.dram_tensor("wqkvT", [D, 3 * FG], BF, kind="ExternalInput").ap()
    woutT = nc.dram_tensor("woutT", [FG, D], BF, kind="ExternalInput").ap()
    mask2 = nc.dram_tensor("mask2", [P, 2, P], BF, kind="ExternalInput").ap()
    seld = nc.dram_tensor("seld", [33, P], F32, kind="ExternalInput").ap()
    out = nc.dram_tensor("out", [T, D], BF, kind="ExternalOutput").ap()
    if debug_out:
        qdbg = nc.dram_tensor("qdbg", [P, 8, T], BF, kind="ExternalOutput").ap()
        vdbg = nc.dram_tensor(
            "vdbg", [P, NTT, HPG, HD + 1], BF, kind="ExternalOutput").ap()
        odbg = nc.dram_tensor("odbg", [P, NPAIR, T], BF, kind="ExternalOutput").ap()
        ddbg = nc.dram_tensor(
            "ddbg", [NIB, NPAIR, 33, 512], F32, kind="ExternalOutput").ap()
        bdbg = nc.dram_tensor(
            "bdbg", [NIB, NPAIR, P, 512], BF, kind="ExternalOutput").ap()

    with tile.TileContext(nc) as tc, ExitStack() as ctx:
        singles = ctx.enter_context(tc.tile_pool(name="singles", bufs=1))
        xtp = ctx.enter_context(tc.tile_pool(name="xt", bufs=2))
        ptp = ctx.enter_context(tc.tile_pool(name="pt", bufs=10))
        bcp = ctx.enter_context(tc.tile_pool(name="bc", bufs=3))
        yp = ctx.enter_context(tc.tile_pool(name="y", bufs=3))
        ps_qk = ctx.enter_context(tc.tile_pool(name="ps_qk", bufs=2, space="PSUM"))
        ps_pv = ctx.enter_context(tc.tile_pool(name="ps_pv", bufs=2, space="PSUM"))
        ps_mm = ctx.enter_context(tc.tile_pool(name="ps_mm", bufs=2, space="PSUM"))

        wq_sb = singles.tile([P, NCC, 3 * FG], BF)
        wq_src = wqkvT.rearrange("(cc p) f -> p cc f", p=P)
        # q/k f-tiles land one DMA each in pair-usage order.  The x block-0
        # DMAs are emitted between ft0/ft4 and the rest (inside
        # emit_qkv_block(0) below) so the first matmul group completes
        # ~5us in; the v slab follows for pair 0's PV.
        for ft in (0, 4):
            nc.sync.dma_start(
                out=wq_sb[:, :, ft * P:(ft + 1) * P],
                in_=wq_src[:, :, ft * P:(ft + 1) * P],
            )

        def dma_wq_rest():
            nc.sync.dma_start(
                out=wq_sb[:, :, 2 * FG:3 * FG], in_=wq_src[:, :, 2 * FG:3 * FG]
            )
            for ft in (1, 5, 2, 6, 3, 7):
                nc.sync.dma_start(
                    out=wq_sb[:, :, ft * P:(ft + 1) * P],
                    in_=wq_src[:, :, ft * P:(ft + 1) * P],
                )
        mask_sb = singles.tile([P, 2, P], BF)
        nc.sync.dma_start(out=mask_sb, in_=mask2)
        sel_sb = singles.tile([33, P], F32)
        nc.sync.dma_start(out=sel_sb, in_=seld)
        wo_sb = singles.tile([P, 4, D], BF)

        qk_sb = singles.tile([P, 8, T], BF)              # f-tiles 0..3 = q, 4..7 = k
        vp_sb = singles.tile([P, NTT, HPG, HD + 1], BF)  # [v_h | ones]
        oT_sb = singles.tile([P, NPAIR, T], BF)          # attn out, [dv-pair, t]
        dp = singles.tile([33, 512], F32)                # den rows 0 (A), 32 (B)
        dpr = singles.tile([33, 512], F32)               # 1/den
        nc.vector.memset(vp_sb[:, :, :, HD:HD + 1], 1.0)
        nc.vector.memset(dp, 1.0)

        # ---- filler thunks: qkv blocks + out-projection ----
        def emit_qkv_block(tb):
            """DMA x-block tb, return one thunk per psum group.
            Order: q/k tiles in pair-usage order, then v tiles."""
            xt = xtp.tile([P, NCC, 512], BF)
            xt_src = xT[:, tb * 512:(tb + 1) * 512].rearrange(
                "(cc p) t -> p cc t", p=P)
            # two half-block DMAs: cheap to issue, and cc 0-3 matmuls can
            # start while cc 4-7 are in flight
            nc.sync.dma_start(out=xt[:, 0:4, :], in_=xt_src[:, 0:4, :])
            nc.sync.dma_start(out=xt[:, 4:8, :], in_=xt_src[:, 4:8, :])
            thunks = []
            for ft in range(8):  # q then k feature tiles, output [f=128, t=512]
                def qk_group(ft=ft, xt=xt, tb=tb):  # key ("qk", tb, ft)
                    ps = ps_mm.tile([P, 512], F32)
                    for cc in range(NCC):
                        nc.tensor.matmul(
                            ps,
                            lhsT=wq_sb[:, cc, ft * P:(ft + 1) * P],
                            rhs=xt[:, cc, :],
                            start=(cc == 0),
                            stop=(cc == NCC - 1),
                        )
                    nc.vector.tensor_copy(
                        out=qk_sb[:, ft, tb * 512:(tb + 1) * 512], in_=ps
                    )
                thunks.append((("qk", tb, ft), qk_group))
            for tl in range(4):  # v in [t, dv] orientation, output [t=128, dv=512]
                def v_group(tl=tl, xt=xt, tb=tb):
                    tt = tb * 4 + tl
                    ps = ps_mm.tile([P, FG], F32)
                    for cc in range(NCC):
                        nc.tensor.matmul(
                            ps,
                            lhsT=xt[:, cc, tl * P:(tl + 1) * P],
                            rhs=wq_sb[:, cc, 2 * FG:3 * FG],
                            start=(cc == 0),
                            stop=(cc == NCC - 1),
                        )
                    nc.vector.tensor_copy(
                        out=vp_sb[:, tt, :, 0:HD],
                        in_=ps.rearrange("p (h d) -> p h d", h=HPG),
                    )
                thunks.append((("v", tb * 4 + tl), v_group))
            # interleave q/k pair-wise: q0,k0,q1,k1,... then v0..v3
            order = [0, 4, 1, 5, 2, 6, 3, 7, 8, 9, 10, 11]
            return [thunks[i] for i in order]

        def emit_outproj_tt(tt, tail=False):
            yt = yp.tile([P, 1024], BF)
            for ob in range(2):
                ps = ps_mm.tile([P, 512], F32, tag="ps", name="ps_op")
                for dc in range(4):
                    nc.tensor.matmul(
                        ps,
                        lhsT=oT_sb[:, dc, tt * P:(tt + 1) * P],
                        rhs=wo_sb[:, dc, ob * 512:(ob + 1) * 512],
                        start=(dc == 0),
                        stop=(dc == 3),
                    )
                if tail:  # ACT is idle in the tail; halve the copy chain
                    nc.scalar.copy(out=yt[:, ob * 512:(ob + 1) * 512], in_=ps)
                else:
                    nc.vector.tensor_copy(yt[:, ob * 512:(ob + 1) * 512], ps)
            nc.sync.dma_start(out=out[tt * P:(tt + 1) * P, :], in_=yt)

        # ---- attention ----
        def emit_attn_pair(ib, m, pop_filler, need, finish_prev):
            """Scores+softmax+PV for head pair (2m, 2m+1), i-block ib.
            pop_filler() emits paced filler; need(keys) force-drains filler
            thunks this pair reads from; finish_prev finishes the previous
            pair's normalization (emitted inside the 64-mode QK runs)."""
            njt = 4 * ib + 4
            isl = slice(ib * 512, (ib + 1) * 512)
            fq, fk = m, 4 + m
            pvA = ps_pv.tile([HD + 1, 512], F32, tag="pv", name="pvA")
            pvB = ps_pv.tile([HD + 1, 512], F32, tag="pv", name="pvB")
            pts = {}

            def qk_run(jts, extra=None):
                for jt in jts:
                    r = jt - 4 * ib
                    c0 = P * r if r > 0 else 0
                    ps = ps_qk.tile([P, 1024], F32)
                    nc.tensor.matmul(
                        ps[:, c0:512],
                        lhsT=qk_sb[0:64, fk, jt * P:(jt + 1) * P],
                        rhs=qk_sb[0:64, fq, ib * 512 + c0:(ib + 1) * 512],
                        start=True, stop=True,
                    )
                    nc.tensor.matmul(
                        ps[:, 512 + c0:1024],
                        lhsT=qk_sb[64:128, fk, jt * P:(jt + 1) * P],
                        rhs=qk_sb[64:128, fq, ib * 512 + c0:(ib + 1) * 512],
                        start=True, stop=True,
                    )
                    if extra is not None:  # same-mode (64,128) deferred work
                        extra(); extra = None
                    pt = ptp.tile([P, 1024], BF)
                    ps2 = ps.rearrange("p (g w) -> p g w", g=2)
                    pt2 = pt.rearrange("p (g w) -> p g w", g=2)
                    nc.scalar.activation(
                        out=pt2[:, :, c0:512], in_=ps2[:, :, c0:512],
                        func=mybir.ActivationFunctionType.Exp, scale=SCALE,
                    )
                    if r >= 0:  # zero the upper triangle of the diagonal block
                        nc.vector.tensor_mul(
                            pt2[:, :, c0:c0 + P], pt2[:, :, c0:c0 + P], mask_sb
                        )
                    pts[jt] = (pt, c0)
                if extra is not None:
                    extra()

            def pv_run(jts):
                for jt in jts:
                    pt, c0 = pts.pop(jt)
                    pt2 = pt.rearrange("p (g w) -> p g w", g=2)
                    first, last = (jt == 0), (jt == njt - 1)
                    nc.tensor.matmul(
                        pvA[:, c0:512],
                        lhsT=vp_sb[:, jt, 2 * m, :],
                        rhs=pt2[:, 0, c0:512],
                        start=first, stop=last, skip_group_check=True,
                    )
                    nc.tensor.matmul(
                        pvB[:, c0:512],
                        lhsT=vp_sb[:, jt, 2 * m + 1, :],
                        rhs=pt2[:, 1, c0:512],
                        start=first, stop=last, skip_group_check=True,
                    )

            need([("qk", tb, fk) for tb in range(ib + 1)] + [("qk", ib, fq)])
            # longer runs in the ACT-bound later blocks: fewer PE mode
            # switches, and the QK->exp chain paces itself there anyway
            rsz = 2 if ib < 2 else 4
            runs = [list(range(s, min(s + rsz, njt)))
                    for s in range(0, njt, rsz)]
            prev = None
            for ri, run in enumerate(runs):
                qk_run(run, extra=(finish_prev if ri == min(1, len(runs) - 1)
                                   else None))
                pop_filler()  # filler before PV: exps + normalize get slack
                if ri <= 1:  # extra filler at the pair boundary, where the
                    pop_filler()  # previous pair's normalization needs slack
                if prev is not None:
                    need([("v", jt) for jt in prev])
                    pv_run(prev)
                prev = run
            need([("v", jt) for jt in prev])
            pv_run(prev)

            # start normalization: gather dens to SBUF (custom DVE ops
            # cannot touch PSUM), then one batched 1/den
            nc.vector.tensor_copy(out=dp[0:1, :], in_=pvA[HD:HD + 1, :])
            nc.vector.tensor_copy(out=dp[32:33, :], in_=pvB[HD:HD + 1, :])
            nc.vector.reciprocal_approx_fast(out=dpr, in_=dp)
            if debug_out:
                nc.sync.dma_start(out=ddbg[ib, m], in_=dpr)

            def finish():
                # broadcast 1/den across partitions with a K=33 selector
                # matmul (64-mode, emitted inside the next pair's QK run),
                # then two fused psum-read muls write normalized oT.
                bc_ps = ps_mm.tile([P, 512], F32, tag="ps", name="bc_ps")
                nc.tensor.matmul(bc_ps, lhsT=sel_sb, rhs=dpr,
                                 start=True, stop=True)
                bc = bcp.tile([P, 512], BF)
                nc.vector.tensor_copy(out=bc, in_=bc_ps)
                if debug_out:
                    nc.sync.dma_start(out=bdbg[ib, m], in_=bc)
                nc.vector.tensor_mul(
                    oT_sb[0:HD, m, isl], pvA[0:HD, :], bc[0:64, :])
                nc.vector.tensor_mul(
                    oT_sb[64:64 + HD, m, isl], pvB[0:HD, :], bc[64:128, :])
            return finish

        # ---- top-level emission ----
        filler = []          # list of (key, fn)
        fill_done = 0
        emitted = set()

        def drain_one():
            nonlocal fill_done
            key, fn = filler[fill_done]
            fn()
            emitted.add(key)
            fill_done += 1

        def need(keys):
            while not all(k in emitted for k in keys):
                drain_one()

        def make_pop(total_runs):
            state = {"run": 0, "base": fill_done}
            def pop():
                state["run"] += 1
                want = min(len(filler), state["base"] + (
                    (len(filler) - state["base"]) * state["run"] // total_runs))
                while fill_done < want:
                    drain_one()
            return pop

        filler += emit_qkv_block(0)   # emits the x block-0 DMAs
        dma_wq_rest()
        # pull the first q/k pair groups upfront so attention starts early
        drain_one(); drain_one()

        finish_prev = None
        for ib in range(NIB):
            if ib == 1:  # wo is needed from ib3; keep it off the hot window
                nc.sync.dma_start(
                    out=wo_sb, in_=woutT.rearrange("(dc p) o -> p dc o", p=P))
            if ib < 3:
                filler.extend(emit_qkv_block(ib + 1))
            else:
                # tts 0-7 read oT of ib0/ib1, fully normalized by now; tts
                # 8-9 (ib2 rows) need finish(ib2, pair3), emitted during
                # pair 0 below, so they join the filler list after it.
                # tts 10-11 are held back as PE work for the final
                # normalization window (they don't depend on pair 3).
                filler.extend(
                    ((("op", tt), (lambda tt=tt: emit_outproj_tt(tt)))
                     for tt in range(8)))
            rsz_ib = 2 if ib < 2 else 4
            npops = -(-(4 * ib + 4) // rsz_ib) + 2  # pops per pair
            pop = make_pop(total_runs=NPAIR * npops)
            for m in range(NPAIR):
                finish_prev = emit_attn_pair(ib, m, pop, need, finish_prev)
                if ib == 3 and m == 0:
                    filler.extend(
                        ((("op", tt), (lambda tt=tt: emit_outproj_tt(tt)))
                         for tt in range(8, 10)))
        emit_outproj_tt(10)
        emit_outproj_tt(11)
        finish_prev()
        while fill_done < len(filler):
            drain_one()
        for tt in range(12, 16):
            emit_outproj_tt(tt, tail=True)
        if debug_out:
            nc.sync.dma_start(out=qdbg, in_=qk_sb)
            nc.sync.dma_start(out=vdbg, in_=vp_sb)
            nc.sync.dma_start(out=odbg, in_=oT_sb)
    nc.compile()
    return nc


def _make_in_maps(x, w_qkv, w_out):
    bf = ml_dtypes.bfloat16
    # 0/1 mask for the diagonal 128x128 block: keep i_local >= j_local
    tri = (np.arange(P)[None, :] >= np.arange(P)[:, None]).astype(np.float32)
    mask2 = np.ascontiguousarray(
        np.broadcast_to(tri[:, None, :], (P, 2, P))).astype(bf)
    sel = np.zeros((33, P), dtype=np.float32)
    sel[0, 0:64] = 1.0
    sel[32, 64:128] = 1.0
    in_maps = []
    for c in range(8):
        b, g = c // 2, c % 2
        wq = w_qkv[g * FG:(g + 1) * FG]
        wk = w_qkv[D + g * FG:D + (g + 1) * FG]
        wv = w_qkv[2 * D + g * FG:2 * D + (g + 1) * FG]
        in_maps.append({
            "xT": np.ascontiguousarray(x[b].T).astype(bf),
            "wqkvT": np.ascontiguousarray(
                np.concatenate([wq.T, wk.T, wv.T], axis=1)).astype(bf),
            "woutT": np.ascontiguousarray(w_out[:, g * FG:(g + 1) * FG].T).astype(bf),
            "mask2": mask2,
            "seld": sel,
        })
    return in_maps


def _ensure_ntff_hook():
    """The agent image's antenv package lacks axon_hooks; shim it so
    run_bass_kernel_spmd(trace=True) can capture NTFF profiles."""
    import sys, types
    try:
        import antenv.axon_hooks  # noqa: F401
        return
    except ImportError:
        pass
    import antenv
    mod = types.ModuleType("antenv.axon_hooks")
    mod._hook = None
    def set_axon_ntff_profile_hook(h):
        mod._hook = h
    def get_axon_ntff_profile_hook():
        return mod._hook
    mod.set_axon_ntff_profile_hook = set_axon_ntff_profile_hook
    mod.get_axon_ntff_profile_hook = get_axon_ntff_profile_hook
    sys.modules["antenv.axon_hooks"] = mod
    antenv.axon_hooks = mod
    try:
        from trn_agent_boot.trn_boot import _ntff_profile_via_ctypes
        set_axon_ntff_profile_hook(
            _ntff_profile_via_ctypes("/opt/axon/libaxon_pjrt.so"))
    except Exception as e:  # degrade to no tracing
        print(f"ntff hook install failed: {e}")


def run(x, w_qkv, w_out, trace=False, trace_kwargs=None, debug=False):
    _import_concourse()
    if trace:
        _ensure_ntff_hook()
    from concourse.bass_utils import run_bass_kernel_spmd

    key = "nc_dbg" if debug else "nc"
    if key not in _CACHE:
        _CACHE[key] = _build_nc(debug_out=debug)
    nc = _CACHE[key]
    in_maps = _make_in_maps(np.asarray(x), np.asarray(w_qkv), np.asarray(w_out))
    kw = dict(trace_kwargs or {})
    res = run_bass_kernel_spmd(nc, in_maps, core_ids=list(range(8)), trace=trace, **kw)
    outs = [r["out"] for r in res.results]
    full = np.empty((B, T, D), dtype=np.float32)
    for b in range(B):
        full[b] = outs[2 * b].astype(np.float32) + outs[2 * b + 1].astype(np.float32)
    return full, res


def kernel(x, w_qkv, w_out):
    full, _ = run(x, w_qkv, w_out, trace=False)
    return full
